# revision 1
# baseline (speedup 1.0000x reference)
"""DiscriminatorStack on 8 trn2 cores.

Sharding: batch-parallel convs (16 img/core), hw-sharded fc + AllGather,
distributed BN stats via tiny AllReduce, bc-sharded minibatch discrimination
(AllGather of feat; each core computes all 128x128 pairs for its 32
mbd channels).
"""
import sys
import numpy as np

sys.path.insert(0, "/opt/trn_rl_repo")
sys.path.insert(0, "/opt/trn_rl_repo/concourse")

import concourse.bass as bass
import concourse.bacc as bacc
import concourse.mybir as mybir
import concourse.tile as tile
from concourse import bass_utils
from concourse.masks import make_identity

dt = mybir.dt
F32 = dt.float32
AF = mybir.ActivationFunctionType
OP = mybir.AluOpType
AX = mybir.AxisListType

R = 8          # cores
NS = 16        # images per core
EPS = 1e-5

_CACHE = {}


def _lrelu_from_psum(nc, sb, psum_ap, out_ap, n_free, tagp):
    """out = lrelu(psum) via r1=relu(x), r2=relu(-x), out = r1 - 0.2*r2."""
    r1 = sb.tile([128, n_free], F32, tag="r1", bufs=2)
    r2 = sb.tile([128, n_free], F32, tag="r2", bufs=2)
    nc.scalar.activation(out=r1[:], in_=psum_ap, func=AF.Relu)
    nc.scalar.activation(out=r2[:], in_=psum_ap, func=AF.Relu, scale=-1.0)
    nc.vector.scalar_tensor_tensor(
        out=out_ap, in0=r2[:], scalar=-0.2, in1=r1[:], op0=OP.mult, op1=OP.add
    )


def _affine_lrelu(nc, sb, x_ap, out_ap, n_free, s_ap, t_ap, ns_ap, nt_ap, tagp):
    """out = lrelu(s*x + t) with per-partition scale/bias APs."""
    r1 = sb.tile([128, n_free], F32, tag="r1", bufs=2)
    r2 = sb.tile([128, n_free], F32, tag="r2", bufs=2)
    nc.scalar.activation(out=r1[:], in_=x_ap, func=AF.Relu, scale=s_ap, bias=t_ap)
    nc.scalar.activation(out=r2[:], in_=x_ap, func=AF.Relu, scale=ns_ap, bias=nt_ap)
    nc.vector.scalar_tensor_tensor(
        out=out_ap, in0=r2[:], scalar=-0.2, in1=r1[:], op0=OP.mult, op1=OP.add
    )


def _bn_finalize(nc, sb, dram, sums, sqs, nblk, n_ch_tiles, n_total, g_t, b_t, name):
    """sums/sqs: [128, n_ch_tiles*nblk] per-block partials. Returns
    (scale, bias, nscale, nbias) [128, n_ch_tiles] after AllReduce."""
    loc = sb.tile([128, 2 * n_ch_tiles], F32, tag=f"bnl{name}")
    nc.vector.tensor_reduce(
        out=loc[:, 0:n_ch_tiles],
        in_=sums[:].rearrange("p (c b) -> p c b", b=nblk),
        axis=AX.X, op=OP.add)
    nc.vector.tensor_reduce(
        out=loc[:, n_ch_tiles:2 * n_ch_tiles],
        in_=sqs[:].rearrange("p (c b) -> p c b", b=nblk),
        axis=AX.X, op=OP.add)
    bn_in = dram.tile([128, 2 * n_ch_tiles], F32)
    bn_out = dram.tile([128, 2 * n_ch_tiles], F32)
    nc.sync.dma_start(bn_in[:], loc[:])
    nc.gpsimd.collective_compute(
        "AllReduce", OP.add, replica_groups=[list(range(R))],
        ins=[bn_in.opt()], outs=[bn_out.opt()])
    glob = sb.tile([128, 2 * n_ch_tiles], F32, tag=f"bng{name}")
    nc.sync.dma_start(glob[:], bn_out[:])
    C = n_ch_tiles
    mean = sb.tile([128, C], F32, tag=f"bnm{name}")
    var = sb.tile([128, C], F32, tag=f"bnv{name}")
    inv = sb.tile([128, C], F32, tag=f"bni{name}")
    scale = sb.tile([128, C], F32, tag=f"bns{name}")
    bias = sb.tile([128, C], F32, tag=f"bnb{name}")
    nscale = sb.tile([128, C], F32, tag=f"bnns{name}")
    nbias = sb.tile([128, C], F32, tag=f"bnnb{name}")
    r = 1.0 / float(n_total)
    nc.vector.tensor_scalar(out=mean[:], in0=glob[:, 0:C], scalar1=r, scalar2=None, op0=OP.mult)
    nc.vector.tensor_scalar(out=var[:], in0=glob[:, C:2 * C], scalar1=r, scalar2=None, op0=OP.mult)
    # var = E[x^2] - mean^2  (in-place safe ops)
    m2 = sb.tile([128, C], F32, tag=f"bnm2{name}")
    nc.vector.tensor_tensor(out=m2[:], in0=mean[:], in1=mean[:], op=OP.mult)
    nc.vector.tensor_tensor(out=var[:], in0=var[:], in1=m2[:], op=OP.subtract)
    nc.vector.tensor_scalar(out=var[:], in0=var[:], scalar1=EPS, scalar2=None, op0=OP.add)
    # sqrt via ACT table + one Newton step (ACT tables are low precision)
    y0 = sb.tile([128, C], F32, tag=f"bny0{name}")
    yr = sb.tile([128, C], F32, tag=f"bnyr{name}")
    nc.scalar.activation(out=y0[:], in_=var[:], func=AF.Sqrt)
    nc.vector.reciprocal(yr[:], y0[:])
    nc.vector.tensor_tensor(out=yr[:], in0=var[:], in1=yr[:], op=OP.mult)
    nc.vector.tensor_tensor(out=y0[:], in0=y0[:], in1=yr[:], op=OP.add)
    nc.vector.tensor_scalar(out=y0[:], in0=y0[:], scalar1=0.5, scalar2=None, op0=OP.mult)
    nc.vector.reciprocal(inv[:], y0[:])
    nc.vector.tensor_tensor(out=scale[:], in0=g_t[:], in1=inv[:], op=OP.mult)
    nc.vector.tensor_tensor(out=bias[:], in0=mean[:], in1=scale[:], op=OP.mult)
    nc.vector.tensor_tensor(out=bias[:], in0=b_t[:], in1=bias[:], op=OP.subtract)
    nc.vector.tensor_scalar(out=nscale[:], in0=scale[:], scalar1=-1.0, scalar2=None, op0=OP.mult)
    nc.vector.tensor_scalar(out=nbias[:], in0=bias[:], scalar1=-1.0, scalar2=None, op0=OP.mult)
    return scale, bias, nscale, nbias


def _build():
    if "nc" in _CACHE:
        return _CACHE["nc"]
    nc = bacc.Bacc("TRN2", target_bir_lowering=False, debug=False, num_devices=R)

    image = nc.dram_tensor("image", [NS, 3, 64, 64], F32, kind="ExternalInput")
    wvT = nc.dram_tensor("wvT", [4096, 128], F32, kind="ExternalInput")
    fcwT = nc.dram_tensor("fcwT", [4096, 512], F32, kind="ExternalInput")
    w1T = nc.dram_tensor("w1T", [4, 4, 4, 128], F32, kind="ExternalInput")
    w2T = nc.dram_tensor("w2T", [128, 16, 256], F32, kind="ExternalInput")
    w3T = nc.dram_tensor("w3T", [2, 128, 16, 256], F32, kind="ExternalInput")
    w4T = nc.dram_tensor("w4T", [4, 2, 128, 16, 128], F32, kind="ExternalInput")
    w5T = nc.dram_tensor("w5T", [2, 4, 128, 16, 128], F32, kind="ExternalInput")
    Tsh = nc.dram_tensor("Tsh", [8, 128, 512], F32, kind="ExternalInput")
    gb = nc.dram_tensor("gb", [128, 16], F32, kind="ExternalInput")
    # gb columns: g2(2) b2(2) g3(2) b3(2) g4(4) b4(4)

    feat_out = nc.dram_tensor("feat_out", [NS, 1024], F32, kind="ExternalOutput")
    mbd_out = nc.dram_tensor("mbd_out", [128, 32], F32, kind="ExternalOutput")

    with tile.TileContext(nc) as tc:
        with (
            tc.tile_pool(name="sb", bufs=1) as sb,
            tc.tile_pool(name="ps", bufs=1, space="PSUM") as ps,
            tc.tile_pool(name="dram", bufs=1, space="DRAM") as dram,
        ):
            ident = sb.tile([128, 128], F32, tag="ident")
            make_identity(nc, ident[:])
            nident = sb.tile([128, 128], F32, tag="nident")
            nc.vector.tensor_scalar(out=nident[:], in0=ident[:], scalar1=-1.0,
                                    scalar2=None, op0=OP.mult)
            gbt = sb.tile([128, 16], F32, tag="gb")
            nc.sync.dma_start(gbt[:], gb.ap())

            # ---------------- fc ----------------
            wvT_sb = sb.tile([128, 32, 128], F32, tag="wvT")
            nc.sync.dma_start(wvT_sb[:], wvT.ap().rearrange("(k p) n -> p k n", p=128))
            pf = ps.tile([128, 512], F32, tag="pb0")
            for kt in range(32):
                fcw_t = sb.tile([128, 512], F32, tag="fcw", bufs=3)
                nc.sync.dma_start(fcw_t[:], fcwT.ap()[kt * 128:(kt + 1) * 128, :])
                nc.tensor.matmul(pf[:], wvT_sb[:, kt, :], fcw_t[:],
                                 start=(kt == 0), stop=(kt == 31))
            wv_sb = sb.tile([128, 512], F32, tag="wvsb")
            nc.scalar.activation(out=wv_sb[:], in_=pf[:], func=AF.Relu)
            ag_fc_in = dram.tile([128, 512], F32)
            ag_fc_out = dram.tile([8, 16, 512], F32)
            nc.sync.dma_start(ag_fc_in[:], wv_sb[:])
            nc.gpsimd.collective_compute(
                "AllToAll", OP.bypass, replica_groups=[list(range(R))],
                ins=[ag_fc_in.opt()], outs=[ag_fc_out.opt()])

            # ---------------- conv1: K=16 (kh,c) contraction, kw accumulation ----------------
            a1_pad = sb.tile([128, 16, 34, 34], F32, tag="bigact")
            nc.gpsimd.memset(a1_pad[:], 0.0)
            w1s = sb.tile([16, 4, 128], F32, tag="w1")
            nc.sync.dma_start(w1s[:], w1T.ap().rearrange("kh c kw m -> (kh c) kw m"))
            agv = ag_fc_out[:].rearrange("r n (h w) -> r n h w", w=64)
            for pair in range(8):
                n0 = pair * 2
                # x0sh[kh*4+c, n, h', w] = xpad[c, n, h'+kh, w], h' = 0..63
                x0 = sb.tile([16, 2, 64, 66], F32, tag="w3", name=f"x0_{pair}")
                nc.gpsimd.memset(x0[:], 0.0)
                wvs = sb.tile([1, 2, 64, 64], F32, tag="actchain", name=f"wvs_{pair}")
                for n in range(2):
                    nc.sync.dma_start(
                        wvs[0:1, n].rearrange("p (a h) w -> p a (h w)", a=8),
                        agv[:, n0 + n].rearrange("a h w -> a (h w)")[None, :, :])
                for kh in range(4):
                    # padded rows h = h'+kh span image rows h-1 = h'+kh-1
                    hp0 = max(0, 1 - kh)            # first h' with valid image row
                    hpn = min(64, 65 - kh) - hp0    # h'+kh-1 <= 63 -> h' <= 64-kh
                    for n in range(2):
                        nc.sync.dma_start(
                            x0[4 * kh:4 * kh + 3, n, hp0:hp0 + hpn, 1:65],
                            image.ap()[n0 + n, 0:3, hp0 + kh - 1:hp0 + kh - 1 + hpn, :])
                        nc.sync.dma_start(
                            x0[4 * kh + 3:4 * kh + 4, n, hp0:hp0 + hpn, 1:65],
                            wvs[0:1, n, hp0 + kh - 1:hp0 + kh - 1 + hpn, :])
                for n in range(2):
                    for hh in range(2):
                        pc = ps.tile([128, 512], F32, tag=f"pb{(n * 2 + hh) % 4}",
                                     name=f"c1p{pair}_{n}_{hh}")
                        for kw in range(4):
                            rhs = x0[:, n, 2 * hh * 16:2 * hh * 16 + 31:2,
                                     kw:kw + 63:2]
                            nc.tensor.matmul(
                                pc[:], w1s[:, kw, :], rhs,
                                start=(kw == 0), stop=(kw == 3))
                        _lrelu_from_psum(
                            nc, sb, pc[:],
                            a1_pad[:, n0 + n, 1 + hh * 16:1 + (hh + 1) * 16, 1:33],
                            512, "c1")

            # ---------------- conv2 ----------------
            w2_sb = sb.tile([128, 16, 256], F32, tag="w2")
            nc.sync.dma_start(w2_sb[:], w2T.ap())
            w3_sb = sb.tile([128, 2, 16, 256], F32, tag="w3")
            nc.sync.dma_start(w3_sb[:], w3T.ap().transpose([1, 0, 2, 3]))
            a2_raw = sb.tile([128, 2, 16, 16, 16], F32, tag="actchain")
            sums2 = sb.tile([128, 16], F32, tag="sums")
            sqs2 = sb.tile([128, 16], F32, tag="sqs")
            scr = sb.tile([128, 512], F32, tag="scr")
            for cot in range(2):
                pbl = [ps.tile([128, 512], F32, tag=f"pb{b}", name=f"c2p{cot}_{b}") for b in range(8)]
                for kk in range(16):
                    lhsT = w2_sb[:, kk, cot * 128:(cot + 1) * 128]
                    kh, kw = kk // 4, kk % 4
                    for blk in range(8):
                        rhs = a1_pad[:, 2 * blk:2 * blk + 2,
                                     kh:kh + 31:2, kw:kw + 31:2]
                        nc.tensor.matmul(pbl[blk][:], lhsT, rhs,
                                         start=(kk == 0), stop=(kk == 15))
                for blk in range(8):
                    nc.scalar.activation(
                        out=a2_raw[:, cot, 2 * blk:2 * blk + 2, :, :],
                        in_=pbl[blk][:], func=AF.Copy,
                        accum_out=sums2[:, cot * 8 + blk:cot * 8 + blk + 1])
                    nc.scalar.activation(
                        out=scr[:], in_=pbl[blk][:], func=AF.Square,
                        accum_out=sqs2[:, cot * 8 + blk:cot * 8 + blk + 1])
            s2, t2, ns2, nt2 = _bn_finalize(
                nc, sb, dram, sums2, sqs2, 8, 2, 128 * 256,
                gbt[:, 0:2], gbt[:, 2:4], "2")
            a2_pad = sb.tile([128, 2, 16, 18, 18], F32, tag="bigact")
            nc.gpsimd.memset(a2_pad[:], 0.0)
            for cot in range(2):
                for i in range(16):
                    _affine_lrelu(
                        nc, sb, a2_raw[:, cot, i, :, :],
                        a2_pad[:, cot, i, 1:17, 1:17], 256,
                        s2[:, cot:cot + 1], t2[:, cot:cot + 1],
                        ns2[:, cot:cot + 1], nt2[:, cot:cot + 1], "n2")

            # ---------------- conv3 ----------------
            a3_raw = sb.tile([128, 2, 16, 8, 8], F32, tag="actchain")
            sums3 = sb.tile([128, 4], F32, tag="sums")
            sqs3 = sb.tile([128, 4], F32, tag="sqs")
            for cot in range(2):
                pbl = [ps.tile([128, 512], F32, tag=f"pb{b}", name=f"c3p{cot}_{b}") for b in range(2)]
                first = True
                for cit in range(2):
                    for kk in range(16):
                        lhsT = w3_sb[:, cit, kk, cot * 128:(cot + 1) * 128]
                        kh, kw = kk // 4, kk % 4
                        for blk in range(2):
                            rhs = a2_pad[:, cit, 8 * blk:8 * blk + 8,
                                         kh:kh + 15:2, kw:kw + 15:2]
                            nc.tensor.matmul(pbl[blk][:], lhsT, rhs,
                                             start=first,
                                             stop=(cit == 1 and kk == 15))
                        first = False
                for blk in range(2):
                    nc.scalar.activation(
                        out=a3_raw[:, cot, 8 * blk:8 * blk + 8, :, :],
                        in_=pbl[blk][:], func=AF.Copy,
                        accum_out=sums3[:, cot * 2 + blk:cot * 2 + blk + 1])
                    nc.scalar.activation(
                        out=scr[:], in_=pbl[blk][:], func=AF.Square,
                        accum_out=sqs3[:, cot * 2 + blk:cot * 2 + blk + 1])
            s3, t3, ns3, nt3 = _bn_finalize(
                nc, sb, dram, sums3, sqs3, 2, 2, 128 * 64,
                gbt[:, 4:6], gbt[:, 6:8], "3")
            a3_pad = sb.tile([128, 2, 16, 10, 10], F32, tag="bigact")
            nc.gpsimd.memset(a3_pad[:], 0.0)
            for cot in range(2):
                for i in range(16):
                    _affine_lrelu(
                        nc, sb, a3_raw[:, cot, i, :, :],
                        a3_pad[:, cot, i, 1:9, 1:9], 64,
                        s3[:, cot:cot + 1], t3[:, cot:cot + 1],
                        ns3[:, cot:cot + 1], nt3[:, cot:cot + 1], "n3")

            # ---------------- conv4 ----------------
            a4_raw = sb.tile([128, 4, 16, 4, 4], F32, tag="actchain")
            sums4 = sb.tile([128, 4], F32, tag="sums")
            sqs4 = sb.tile([128, 4], F32, tag="sqs")
            for cot in range(4):
                w4c = sb.tile([128, 2, 16, 128], F32, tag="w2")
                nc.sync.dma_start(
                    w4c[:], w4T.ap()[cot].transpose([1, 0, 2, 3]))
                pb = ps.tile([128, 256], F32, tag="pb0")
                first = True
                for cit in range(2):
                    for kk in range(16):
                        kh, kw = kk // 4, kk % 4
                        rhs = a3_pad[:, cit, :, kh:kh + 7:2, kw:kw + 7:2]
                        nc.tensor.matmul(pb[:], w4c[:, cit, kk, :], rhs,
                                         start=first,
                                         stop=(cit == 1 and kk == 15))
                        first = False
                nc.scalar.activation(
                    out=a4_raw[:, cot, :, :, :], in_=pb[:], func=AF.Copy,
                    accum_out=sums4[:, cot:cot + 1])
                nc.scalar.activation(
                    out=scr[:, 0:256], in_=pb[:], func=AF.Square,
                    accum_out=sqs4[:, cot:cot + 1])
            s4, t4, ns4, nt4 = _bn_finalize(
                nc, sb, dram, sums4, sqs4, 1, 4, 128 * 16,
                gbt[:, 8:12], gbt[:, 12:16], "4")
            a4_pad = sb.tile([128, 4, 16, 6, 6], F32, tag="bigact")
            nc.gpsimd.memset(a4_pad[:], 0.0)
            for cot in range(4):
                for i in range(16):
                    _affine_lrelu(
                        nc, sb, a4_raw[:, cot, i, :, :],
                        a4_pad[:, cot, i, 1:5, 1:5], 16,
                        s4[:, cot:cot + 1], t4[:, cot:cot + 1],
                        ns4[:, cot:cot + 1], nt4[:, cot:cot + 1], "n4")

            # ---------------- conv5 -> feat ----------------
            a5_sb = sb.tile([128, 2, 16, 2, 2], F32, tag="a5")
            for cot in range(2):
                w5c = sb.tile([128, 4, 16, 128], F32, tag="w3")
                nc.sync.dma_start(
                    w5c[:], w5T.ap()[cot].transpose([1, 0, 2, 3]))
                pb = ps.tile([128, 64], F32, tag="pb1")
                first = True
                for cit in range(4):
                    for kk in range(16):
                        kh, kw = kk // 4, kk % 4
                        rhs = a4_pad[:, cit, :, kh:kh + 3:2, kw:kw + 3:2]
                        nc.tensor.matmul(pb[:], w5c[:, cit, kk, :], rhs,
                                         start=first,
                                         stop=(cit == 3 and kk == 15))
                        first = False
                nc.scalar.copy(a5_sb[:, cot, :, :, :], pb[:])
            ftl = dram.tile([16, 1024], F32)
            ag_ft = dram.tile([8, 16, 1024], F32)
            for ct in range(2):
                fo_dst = feat_out.ap().rearrange(
                    "n (ct c s) -> ct c n s", ct=2, s=4)[ct]
                src_ap = a5_sb[:, ct, :, :, :].rearrange("c n h w -> c n (h w)")
                nc.sync.dma_start(fo_dst, src_ap)
                fl_dst = ftl[:].rearrange("n (ct c s) -> ct c n s", ct=2, s=4)[ct]
                nc.sync.dma_start(fl_dst, src_ap)
            nc.gpsimd.collective_compute(
                "AllGather", OP.bypass, replica_groups=[list(range(R))],
                ins=[ftl.opt()], outs=[ag_ft.opt()])

            # ---------------- Ms = featT.T @ T_shard ----------------
            T_sb = sb.tile([128, 8, 512], F32, tag="w3")
            nc.sync.dma_start(T_sb[:], Tsh.ap().transpose([1, 0, 2]))
            featn = sb.tile([128, 8, 128], F32, tag="w2")
            nc.sync.dma_start(
                featn[:],
                ag_ft[:].rearrange("r n (at a) -> (r n) at a", at=8))
            fts = []
            for at in range(8):
                ptr = ps.tile([128, 128], F32, tag=f"pb{at % 4}", name=f"ptr{at}")
                nc.tensor.transpose(ptr[:], featn[:, at, :], ident[:])
                ft = sb.tile([128, 128], F32, tag=f"ft{at}", name=f"ftt{at}")
                nc.scalar.copy(ft[:], ptr[:])
                fts.append(ft)
            pms = ps.tile([128, 512], F32, tag="pb4")
            for at in range(8):
                nc.tensor.matmul(
                    pms[:], fts[at][:], T_sb[:, at, :],
                    start=(at == 0), stop=(at == 7))
            ms_p = sb.tile([128, 512], F32, tag="msp")
            nc.scalar.copy(ms_p[:], pms[:])

            # ---------------- pairwise L1 + exp + sum ----------------
            exp_all = sb.tile([128, 32, 128], F32, tag="wvT")
            for jc in range(32):
                base = 4 * (jc % 2)
                dch = sb.tile([128, 4, 32], F32, tag="dch", bufs=2)
                for jj in range(4):
                    j = jc * 4 + jj
                    pbj = ps.tile([128, 512], F32, tag=f"pb{base + jj if base else jj}",
                                  name=f"pw{jc}_{jj}")
                    nc.tensor.matmul(
                        pbj[:], ident[:, j:j + 1].broadcast_to([128, 128]),
                        ms_p[:], start=True, stop=False)
                    nc.tensor.matmul(
                        pbj[:], nident[:], ms_p[:], start=False, stop=True)
                    nc.vector.tensor_reduce(
                        out=dch[:, jj, :],
                        in_=pbj[:].rearrange("p (b c) -> p b c", c=16),
                        axis=AX.X, op=OP.add, apply_absolute_value=True)
                nc.scalar.activation(
                    out=exp_all[:, :, jc * 4:(jc + 1) * 4].transpose([0, 2, 1]),
                    in_=dch[:], func=AF.Exp, scale=-1.0)
            mbd_sb = sb.tile([128, 32], F32, tag="mbd")
            nc.vector.tensor_reduce(
                out=mbd_sb[:], in_=exp_all[:], axis=AX.X, op=OP.add)
            nc.sync.dma_start(mbd_out.ap(), mbd_sb[:])

    nc.compile()
    _CACHE["nc"] = nc
    return nc


def _prep_in_maps(inputs):
    image = np.asarray(inputs["image"], np.float32)
    wv = np.asarray(inputs["word_vectors"], np.float32)
    fc_w = np.asarray(inputs["fc_w"], np.float32)
    w1 = np.asarray(inputs["w1"], np.float32)
    w2 = np.asarray(inputs["w2"], np.float32)
    w3 = np.asarray(inputs["w3"], np.float32)
    w4 = np.asarray(inputs["w4"], np.float32)
    w5 = np.asarray(inputs["w5"], np.float32)
    T = np.asarray(inputs["T"], np.float32).reshape(1024, 4096)

    wvT = np.ascontiguousarray(wv.T)
    fcwT = np.ascontiguousarray(fc_w.T)
    w1T = np.ascontiguousarray(w1.transpose(2, 1, 3, 0))  # [kh, c, kw, cout]
    w2T = np.ascontiguousarray(w2.transpose(1, 2, 3, 0).reshape(128, 16, 256))
    w3T = np.ascontiguousarray(w3.transpose(1, 2, 3, 0).reshape(2, 128, 16, 256))
    w4T = np.ascontiguousarray(w4.transpose(1, 2, 3, 0).reshape(2, 128, 16, 4, 128).transpose(3, 0, 1, 2, 4))
    w5T = np.ascontiguousarray(w5.transpose(1, 2, 3, 0).reshape(4, 128, 16, 2, 128).transpose(3, 0, 1, 2, 4))

    def gbvec(g, b, n_t):
        out = np.zeros((128, 2 * n_t), np.float32)
        out[:, :n_t] = g.reshape(n_t, 128).T
        out[:, n_t:] = b.reshape(n_t, 128).T
        return out

    gb = np.zeros((128, 16), np.float32)
    gb[:, 0:4] = gbvec(np.asarray(inputs["g2"], np.float32), np.asarray(inputs["b2"], np.float32), 2)
    gb[:, 4:8] = gbvec(np.asarray(inputs["g3"], np.float32), np.asarray(inputs["b3"], np.float32), 2)
    gb[:, 8:16] = gbvec(np.asarray(inputs["g4"], np.float32), np.asarray(inputs["b4"], np.float32), 4)

    in_maps = []
    for r in range(R):
        in_maps.append({
            "image": np.ascontiguousarray(image[r * NS:(r + 1) * NS]),
            "wvT": wvT,
            "fcwT": np.ascontiguousarray(fcwT[:, r * 512:(r + 1) * 512]),
            "w1T": w1T, "w2T": w2T, "w3T": w3T, "w4T": w4T, "w5T": w5T,
            "Tsh": np.ascontiguousarray(T[:, r * 512:(r + 1) * 512].reshape(8, 128, 512)),
            "gb": gb,
        })
    return in_maps


def kernel(**inputs) -> np.ndarray:
    nc = _build()
    in_maps = _prep_in_maps(inputs)
    res = bass_utils.run_bass_kernel_spmd(nc, in_maps, core_ids=list(range(R)))
    feat = np.concatenate([res.results[r]["feat_out"] for r in range(R)], axis=0)
    mbd = np.concatenate([res.results[r]["mbd_out"] for r in range(R)], axis=1)
    return np.concatenate([feat, mbd], axis=1).astype(np.float32)


if __name__ == "__main__":
    _build()
    print("build ok")



# revision 2
# speedup vs baseline: 52.0274x; 52.0274x over previous
"""DiscriminatorStack on 8 trn2 cores.

Sharding: batch-parallel convs (16 img/core), hw-sharded fc + AllGather,
distributed BN stats via tiny AllReduce, bc-sharded minibatch discrimination
(AllGather of feat; each core computes all 128x128 pairs for its 32
mbd channels).
"""
import sys
import numpy as np

sys.path.insert(0, "/opt/trn_rl_repo")
sys.path.insert(0, "/opt/trn_rl_repo/concourse")

import concourse.bass as bass
import concourse.bacc as bacc
import concourse.mybir as mybir
import concourse.tile as tile
from concourse import bass_utils
from concourse.masks import make_identity

dt = mybir.dt
F32 = dt.float32
F32R = dt.float32r


def _mm(nc, out, lhsT, rhs, **kw):
    # float32r: 1 PE cycle/row (vs 4 for fp32) at free-size >= 256
    nc.tensor.matmul(out, lhsT.bitcast(F32R), rhs.bitcast(F32R), **kw)
AF = mybir.ActivationFunctionType
OP = mybir.AluOpType
AX = mybir.AxisListType

R = 8          # cores
NS = 16        # images per core
EPS = 1e-5

_CACHE = {}


def _lrelu_from_psum(nc, sb, psum_ap, out_ap, n_free, tagp):
    """out = lrelu(psum) via r1=relu(x), r2=relu(-x), out = r1 - 0.2*r2."""
    r1 = sb.tile([128, n_free], F32, tag="r1", bufs=2)
    r2 = sb.tile([128, n_free], F32, tag="r2", bufs=2)
    nc.scalar.activation(out=r1[:], in_=psum_ap, func=AF.Relu)
    nc.scalar.activation(out=r2[:], in_=psum_ap, func=AF.Relu, scale=-1.0)
    nc.vector.scalar_tensor_tensor(
        out=out_ap, in0=r2[:], scalar=-0.2, in1=r1[:], op0=OP.mult, op1=OP.add
    )


def _affine_lrelu(nc, sb, x_ap, out_ap, n_free, s_ap, t_ap, ns_ap, nt_ap, tagp):
    """out = lrelu(s*x + t) with per-partition scale/bias APs."""
    r1 = sb.tile([128, n_free], F32, tag="r1", bufs=2)
    r2 = sb.tile([128, n_free], F32, tag="r2", bufs=2)
    nc.scalar.activation(out=r1[:], in_=x_ap, func=AF.Relu, scale=s_ap, bias=t_ap)
    nc.scalar.activation(out=r2[:], in_=x_ap, func=AF.Relu, scale=ns_ap, bias=nt_ap)
    nc.vector.scalar_tensor_tensor(
        out=out_ap, in0=r2[:], scalar=-0.2, in1=r1[:], op0=OP.mult, op1=OP.add
    )


def _bn_finalize(nc, sb, dram, sums, sqs, nblk, n_ch_tiles, n_total, g_t, b_t, name):
    """sums/sqs: [128, n_ch_tiles*nblk] per-block partials. Returns
    (scale, bias, nscale, nbias) [128, n_ch_tiles] after AllReduce."""
    loc = sb.tile([128, 2 * n_ch_tiles], F32, tag=f"bnl{name}")
    nc.vector.tensor_reduce(
        out=loc[:, 0:n_ch_tiles],
        in_=sums[:].rearrange("p (c b) -> p c b", b=nblk),
        axis=AX.X, op=OP.add)
    nc.vector.tensor_reduce(
        out=loc[:, n_ch_tiles:2 * n_ch_tiles],
        in_=sqs[:].rearrange("p (c b) -> p c b", b=nblk),
        axis=AX.X, op=OP.add)
    bn_in = dram.tile([128, 2 * n_ch_tiles], F32)
    bn_out = dram.tile([128, 2 * n_ch_tiles], F32)
    nc.sync.dma_start(bn_in[:], loc[:])
    nc.gpsimd.collective_compute(
        "AllReduce", OP.add, replica_groups=[list(range(R))],
        ins=[bn_in.opt()], outs=[bn_out.opt()])
    glob = sb.tile([128, 2 * n_ch_tiles], F32, tag=f"bng{name}")
    nc.sync.dma_start(glob[:], bn_out[:])
    C = n_ch_tiles
    mean = sb.tile([128, C], F32, tag=f"bnm{name}")
    var = sb.tile([128, C], F32, tag=f"bnv{name}")
    inv = sb.tile([128, C], F32, tag=f"bni{name}")
    scale = sb.tile([128, C], F32, tag=f"bns{name}")
    bias = sb.tile([128, C], F32, tag=f"bnb{name}")
    nscale = sb.tile([128, C], F32, tag=f"bnns{name}")
    nbias = sb.tile([128, C], F32, tag=f"bnnb{name}")
    r = 1.0 / float(n_total)
    nc.vector.tensor_scalar(out=mean[:], in0=glob[:, 0:C], scalar1=r, scalar2=None, op0=OP.mult)
    nc.vector.tensor_scalar(out=var[:], in0=glob[:, C:2 * C], scalar1=r, scalar2=None, op0=OP.mult)
    # var = E[x^2] - mean^2  (in-place safe ops)
    m2 = sb.tile([128, C], F32, tag=f"bnm2{name}")
    nc.vector.tensor_tensor(out=m2[:], in0=mean[:], in1=mean[:], op=OP.mult)
    nc.vector.tensor_tensor(out=var[:], in0=var[:], in1=m2[:], op=OP.subtract)
    nc.vector.tensor_scalar(out=var[:], in0=var[:], scalar1=EPS, scalar2=None, op0=OP.add)
    # sqrt via ACT table + one Newton step (ACT tables are low precision)
    y0 = sb.tile([128, C], F32, tag=f"bny0{name}")
    yr = sb.tile([128, C], F32, tag=f"bnyr{name}")
    nc.scalar.activation(out=y0[:], in_=var[:], func=AF.Sqrt)
    nc.vector.reciprocal(yr[:], y0[:])
    nc.vector.tensor_tensor(out=yr[:], in0=var[:], in1=yr[:], op=OP.mult)
    nc.vector.tensor_tensor(out=y0[:], in0=y0[:], in1=yr[:], op=OP.add)
    nc.vector.tensor_scalar(out=y0[:], in0=y0[:], scalar1=0.5, scalar2=None, op0=OP.mult)
    nc.vector.reciprocal(inv[:], y0[:])
    nc.vector.tensor_tensor(out=scale[:], in0=g_t[:], in1=inv[:], op=OP.mult)
    nc.vector.tensor_tensor(out=bias[:], in0=mean[:], in1=scale[:], op=OP.mult)
    nc.vector.tensor_tensor(out=bias[:], in0=b_t[:], in1=bias[:], op=OP.subtract)
    nc.vector.tensor_scalar(out=nscale[:], in0=scale[:], scalar1=-1.0, scalar2=None, op0=OP.mult)
    nc.vector.tensor_scalar(out=nbias[:], in0=bias[:], scalar1=-1.0, scalar2=None, op0=OP.mult)
    return scale, bias, nscale, nbias


def _build():
    if "nc" in _CACHE:
        return _CACHE["nc"]
    nc = bacc.Bacc("TRN2", target_bir_lowering=False, debug=False, num_devices=R)

    image = nc.dram_tensor("image", [NS, 3, 64, 64], F32, kind="ExternalInput")
    wvT = nc.dram_tensor("wvT", [4096, 128], F32, kind="ExternalInput")
    fcwT = nc.dram_tensor("fcwT", [4096, 512], F32, kind="ExternalInput")
    w1T = nc.dram_tensor("w1T", [4, 4, 4, 128], F32, kind="ExternalInput")
    w2T = nc.dram_tensor("w2T", [128, 16, 256], F32, kind="ExternalInput")
    w3T = nc.dram_tensor("w3T", [2, 128, 16, 256], F32, kind="ExternalInput")
    w4T = nc.dram_tensor("w4T", [4, 2, 128, 16, 128], F32, kind="ExternalInput")
    w5T = nc.dram_tensor("w5T", [2, 4, 128, 16, 128], F32, kind="ExternalInput")
    Tsh = nc.dram_tensor("Tsh", [8, 128, 512], F32, kind="ExternalInput")
    gb = nc.dram_tensor("gb", [128, 16], F32, kind="ExternalInput")
    # gb columns: g2(2) b2(2) g3(2) b3(2) g4(4) b4(4)

    feat_out = nc.dram_tensor("feat_out", [NS, 1024], F32, kind="ExternalOutput")
    mbd_out = nc.dram_tensor("mbd_out", [128, 32], F32, kind="ExternalOutput")

    with tile.TileContext(nc) as tc:
        with (
            tc.tile_pool(name="sb", bufs=1) as sb,
            tc.tile_pool(name="ps", bufs=1, space="PSUM") as ps,
            tc.tile_pool(name="dram", bufs=1, space="DRAM") as dram,
        ):
            ident = sb.tile([128, 128], F32, tag="ident")
            make_identity(nc, ident[:])
            nident = sb.tile([128, 128], F32, tag="nident")
            nc.vector.tensor_scalar(out=nident[:], in0=ident[:], scalar1=-1.0,
                                    scalar2=None, op0=OP.mult)
            gbt = sb.tile([128, 16], F32, tag="gb")
            nc.sync.dma_start(gbt[:], gb.ap())

            # ---------------- fc ----------------
            wvT_sb = sb.tile([128, 32, 128], F32, tag="wvT")
            nc.sync.dma_start(wvT_sb[:], wvT.ap().rearrange("(k p) n -> p k n", p=128))
            pf = ps.tile([128, 512], F32, tag="pb0")
            for kt in range(32):
                fcw_t = sb.tile([128, 512], F32, tag="fcw", bufs=3)
                nc.sync.dma_start(fcw_t[:], fcwT.ap()[kt * 128:(kt + 1) * 128, :])
                _mm(nc, pf[:], wvT_sb[:, kt, :], fcw_t[:],
                    start=(kt == 0), stop=(kt == 31))
            wv_sb = sb.tile([128, 512], F32, tag="wvsb")
            nc.scalar.activation(out=wv_sb[:], in_=pf[:], func=AF.Relu)
            ag_fc_in = dram.tile([128, 512], F32)
            ag_fc_out = dram.tile([8, 16, 512], F32)
            nc.sync.dma_start(ag_fc_in[:], wv_sb[:])
            nc.gpsimd.collective_compute(
                "AllToAll", OP.bypass, replica_groups=[list(range(R))],
                ins=[ag_fc_in.opt()], outs=[ag_fc_out.opt()])

            # ---------------- conv1: K=16 (kh,c) contraction, kw accumulation ----------------
            a1_pad = sb.tile([128, 16, 34, 34], F32, tag="bigact")
            nc.gpsimd.memset(a1_pad[:], 0.0)
            w1s = sb.tile([16, 4, 128], F32, tag="w1")
            nc.sync.dma_start(w1s[:], w1T.ap().rearrange("kh c kw m -> (kh c) kw m"))
            agv = ag_fc_out[:].rearrange("r n (h w) -> r n h w", w=64)
            for pair in range(8):
                n0 = pair * 2
                # x0sh[kh*4+c, n, h', w] = xpad[c, n, h'+kh, w], h' = 0..63
                x0 = sb.tile([16, 2, 64, 66], F32, tag="w3", name=f"x0_{pair}")
                nc.gpsimd.memset(x0[:], 0.0)
                wvs = sb.tile([1, 2, 64, 64], F32, tag="actchain", name=f"wvs_{pair}")
                for n in range(2):
                    nc.sync.dma_start(
                        wvs[0:1, n].rearrange("p (a h) w -> p a (h w)", a=8),
                        agv[:, n0 + n].rearrange("a h w -> a (h w)")[None, :, :])
                for kh in range(4):
                    # padded rows h = h'+kh span image rows h-1 = h'+kh-1
                    hp0 = max(0, 1 - kh)            # first h' with valid image row
                    hpn = min(64, 65 - kh) - hp0    # h'+kh-1 <= 63 -> h' <= 64-kh
                    for n in range(2):
                        nc.sync.dma_start(
                            x0[4 * kh:4 * kh + 3, n, hp0:hp0 + hpn, 1:65],
                            image.ap()[n0 + n, 0:3, hp0 + kh - 1:hp0 + kh - 1 + hpn, :])
                        nc.sync.dma_start(
                            x0[4 * kh + 3:4 * kh + 4, n, hp0:hp0 + hpn, 1:65],
                            wvs[0:1, n, hp0 + kh - 1:hp0 + kh - 1 + hpn, :])
                for n in range(2):
                    for hh in range(2):
                        pc = ps.tile([128, 512], F32, tag=f"pb{(n * 2 + hh) % 4}",
                                     name=f"c1p{pair}_{n}_{hh}")
                        for kw in range(4):
                            rhs = x0[:, n, 2 * hh * 16:2 * hh * 16 + 31:2,
                                     kw:kw + 63:2]
                            _mm(nc, pc[:], w1s[:, kw, :], rhs,
                                start=(kw == 0), stop=(kw == 3))
                        _lrelu_from_psum(
                            nc, sb, pc[:],
                            a1_pad[:, n0 + n, 1 + hh * 16:1 + (hh + 1) * 16, 1:33],
                            512, "c1")

            # ---------------- conv2 ----------------
            w2_sb = sb.tile([128, 16, 256], F32, tag="w2")
            nc.sync.dma_start(w2_sb[:], w2T.ap())
            w3_sb = sb.tile([128, 2, 16, 256], F32, tag="w3")
            nc.sync.dma_start(w3_sb[:], w3T.ap().transpose([1, 0, 2, 3]))
            a2_raw = sb.tile([128, 2, 16, 16, 16], F32, tag="actchain")
            sums2 = sb.tile([128, 16], F32, tag="sums")
            sqs2 = sb.tile([128, 16], F32, tag="sqs")
            scr = sb.tile([128, 512], F32, tag="scr")
            for cot in range(2):
                pbl = [ps.tile([128, 512], F32, tag=f"pb{b}", name=f"c2p{cot}_{b}") for b in range(8)]
                for kk in range(16):
                    lhsT = w2_sb[:, kk, cot * 128:(cot + 1) * 128]
                    kh, kw = kk // 4, kk % 4
                    for blk in range(8):
                        rhs = a1_pad[:, 2 * blk:2 * blk + 2,
                                     kh:kh + 31:2, kw:kw + 31:2]
                        _mm(nc, pbl[blk][:], lhsT, rhs,
                            start=(kk == 0), stop=(kk == 15))
                for blk in range(8):
                    nc.scalar.activation(
                        out=a2_raw[:, cot, 2 * blk:2 * blk + 2, :, :],
                        in_=pbl[blk][:], func=AF.Copy,
                        accum_out=sums2[:, cot * 8 + blk:cot * 8 + blk + 1])
                    nc.scalar.activation(
                        out=scr[:], in_=pbl[blk][:], func=AF.Square,
                        accum_out=sqs2[:, cot * 8 + blk:cot * 8 + blk + 1])
            s2, t2, ns2, nt2 = _bn_finalize(
                nc, sb, dram, sums2, sqs2, 8, 2, 128 * 256,
                gbt[:, 0:2], gbt[:, 2:4], "2")
            a2_pad = sb.tile([128, 2, 16, 18, 18], F32, tag="bigact")
            nc.gpsimd.memset(a2_pad[:], 0.0)
            for cot in range(2):
                for i in range(16):
                    _affine_lrelu(
                        nc, sb, a2_raw[:, cot, i, :, :],
                        a2_pad[:, cot, i, 1:17, 1:17], 256,
                        s2[:, cot:cot + 1], t2[:, cot:cot + 1],
                        ns2[:, cot:cot + 1], nt2[:, cot:cot + 1], "n2")

            # ---------------- conv3 ----------------
            a3_raw = sb.tile([128, 2, 16, 8, 8], F32, tag="actchain")
            sums3 = sb.tile([128, 4], F32, tag="sums")
            sqs3 = sb.tile([128, 4], F32, tag="sqs")
            for cot in range(2):
                pbl = [ps.tile([128, 512], F32, tag=f"pb{b}", name=f"c3p{cot}_{b}") for b in range(2)]
                first = True
                for cit in range(2):
                    for kk in range(16):
                        lhsT = w3_sb[:, cit, kk, cot * 128:(cot + 1) * 128]
                        kh, kw = kk // 4, kk % 4
                        for blk in range(2):
                            rhs = a2_pad[:, cit, 8 * blk:8 * blk + 8,
                                         kh:kh + 15:2, kw:kw + 15:2]
                            _mm(nc, pbl[blk][:], lhsT, rhs,
                                start=first,
                                stop=(cit == 1 and kk == 15))
                        first = False
                for blk in range(2):
                    nc.scalar.activation(
                        out=a3_raw[:, cot, 8 * blk:8 * blk + 8, :, :],
                        in_=pbl[blk][:], func=AF.Copy,
                        accum_out=sums3[:, cot * 2 + blk:cot * 2 + blk + 1])
                    nc.scalar.activation(
                        out=scr[:], in_=pbl[blk][:], func=AF.Square,
                        accum_out=sqs3[:, cot * 2 + blk:cot * 2 + blk + 1])
            s3, t3, ns3, nt3 = _bn_finalize(
                nc, sb, dram, sums3, sqs3, 2, 2, 128 * 64,
                gbt[:, 4:6], gbt[:, 6:8], "3")
            a3_pad = sb.tile([128, 2, 16, 10, 10], F32, tag="bigact")
            nc.gpsimd.memset(a3_pad[:], 0.0)
            for cot in range(2):
                for i in range(16):
                    _affine_lrelu(
                        nc, sb, a3_raw[:, cot, i, :, :],
                        a3_pad[:, cot, i, 1:9, 1:9], 64,
                        s3[:, cot:cot + 1], t3[:, cot:cot + 1],
                        ns3[:, cot:cot + 1], nt3[:, cot:cot + 1], "n3")

            # ---------------- conv4 ----------------
            a4_raw = sb.tile([128, 4, 16, 4, 4], F32, tag="actchain")
            sums4 = sb.tile([128, 4], F32, tag="sums")
            sqs4 = sb.tile([128, 4], F32, tag="sqs")
            for cot in range(4):
                w4c = sb.tile([128, 2, 16, 128], F32, tag="w2")
                nc.sync.dma_start(
                    w4c[:], w4T.ap()[cot].transpose([1, 0, 2, 3]))
                pb = ps.tile([128, 256], F32, tag="pb0")
                first = True
                for cit in range(2):
                    for kk in range(16):
                        kh, kw = kk // 4, kk % 4
                        rhs = a3_pad[:, cit, :, kh:kh + 7:2, kw:kw + 7:2]
                        _mm(nc, pb[:], w4c[:, cit, kk, :], rhs,
                            start=first,
                            stop=(cit == 1 and kk == 15))
                        first = False
                nc.scalar.activation(
                    out=a4_raw[:, cot, :, :, :], in_=pb[:], func=AF.Copy,
                    accum_out=sums4[:, cot:cot + 1])
                nc.scalar.activation(
                    out=scr[:, 0:256], in_=pb[:], func=AF.Square,
                    accum_out=sqs4[:, cot:cot + 1])
            s4, t4, ns4, nt4 = _bn_finalize(
                nc, sb, dram, sums4, sqs4, 1, 4, 128 * 16,
                gbt[:, 8:12], gbt[:, 12:16], "4")
            a4_pad = sb.tile([128, 4, 16, 6, 6], F32, tag="bigact")
            nc.gpsimd.memset(a4_pad[:], 0.0)
            for cot in range(4):
                for i in range(16):
                    _affine_lrelu(
                        nc, sb, a4_raw[:, cot, i, :, :],
                        a4_pad[:, cot, i, 1:5, 1:5], 16,
                        s4[:, cot:cot + 1], t4[:, cot:cot + 1],
                        ns4[:, cot:cot + 1], nt4[:, cot:cot + 1], "n4")

            # ---------------- conv5 -> feat ----------------
            a5_sb = sb.tile([128, 2, 16, 2, 2], F32, tag="a5")
            for cot in range(2):
                w5c = sb.tile([128, 4, 16, 128], F32, tag="w3")
                nc.sync.dma_start(
                    w5c[:], w5T.ap()[cot].transpose([1, 0, 2, 3]))
                pb = ps.tile([128, 64], F32, tag="pb1")
                first = True
                for cit in range(4):
                    for kk in range(16):
                        kh, kw = kk // 4, kk % 4
                        rhs = a4_pad[:, cit, :, kh:kh + 3:2, kw:kw + 3:2]
                        _mm(nc, pb[:], w5c[:, cit, kk, :], rhs,
                            start=first,
                            stop=(cit == 3 and kk == 15))
                        first = False
                nc.scalar.copy(a5_sb[:, cot, :, :, :], pb[:])
            ftl = dram.tile([16, 1024], F32)
            ag_ft = dram.tile([8, 16, 1024], F32)
            for ct in range(2):
                fo_dst = feat_out.ap().rearrange(
                    "n (ct c s) -> ct c n s", ct=2, s=4)[ct]
                src_ap = a5_sb[:, ct, :, :, :].rearrange("c n h w -> c n (h w)")
                nc.sync.dma_start(fo_dst, src_ap)
                fl_dst = ftl[:].rearrange("n (ct c s) -> ct c n s", ct=2, s=4)[ct]
                nc.sync.dma_start(fl_dst, src_ap)
            nc.gpsimd.collective_compute(
                "AllGather", OP.bypass, replica_groups=[list(range(R))],
                ins=[ftl.opt()], outs=[ag_ft.opt()])

            # ---------------- Ms = featT.T @ T_shard ----------------
            T_sb = sb.tile([128, 8, 512], F32, tag="w3")
            nc.sync.dma_start(T_sb[:], Tsh.ap().transpose([1, 0, 2]))
            featn = sb.tile([128, 8, 128], F32, tag="w2")
            nc.sync.dma_start(
                featn[:],
                ag_ft[:].rearrange("r n (at a) -> (r n) at a", at=8))
            fts = []
            for at in range(8):
                ptr = ps.tile([128, 128], F32, tag=f"pb{at % 4}", name=f"ptr{at}")
                nc.tensor.transpose(ptr[:], featn[:, at, :], ident[:])
                ft = sb.tile([128, 128], F32, tag=f"ft{at}", name=f"ftt{at}")
                nc.scalar.copy(ft[:], ptr[:])
                fts.append(ft)
            pms = ps.tile([128, 512], F32, tag="pb4")
            for at in range(8):
                _mm(nc, pms[:], fts[at][:], T_sb[:, at, :],
                    start=(at == 0), stop=(at == 7))
            ms_p = sb.tile([128, 512], F32, tag="msp")
            nc.scalar.copy(ms_p[:], pms[:])

            # ---------------- pairwise L1 + exp + sum ----------------
            exp_all = sb.tile([128, 32, 128], F32, tag="wvT")
            for jc in range(32):
                base = 4 * (jc % 2)
                dch = sb.tile([128, 4, 32], F32, tag="dch", bufs=2)
                for jj in range(4):
                    j = jc * 4 + jj
                    pbj = ps.tile([128, 512], F32, tag=f"pb{base + jj if base else jj}",
                                  name=f"pw{jc}_{jj}")
                    _mm(nc, pbj[:], ident[:, j:j + 1].broadcast_to([128, 128]),
                        ms_p[:], start=True, stop=False)
                    _mm(nc, pbj[:], nident[:], ms_p[:], start=False, stop=True)
                    nc.vector.tensor_reduce(
                        out=dch[:, jj, :],
                        in_=pbj[:].rearrange("p (b c) -> p b c", c=16),
                        axis=AX.X, op=OP.add, apply_absolute_value=True)
                nc.scalar.activation(
                    out=exp_all[:, :, jc * 4:(jc + 1) * 4].transpose([0, 2, 1]),
                    in_=dch[:], func=AF.Exp, scale=-1.0)
            mbd_sb = sb.tile([128, 32], F32, tag="mbd")
            nc.vector.tensor_reduce(
                out=mbd_sb[:], in_=exp_all[:], axis=AX.X, op=OP.add)
            nc.sync.dma_start(mbd_out.ap(), mbd_sb[:])

    nc.compile()
    _CACHE["nc"] = nc
    return nc


def _prep_in_maps(inputs):
    image = np.asarray(inputs["image"], np.float32)
    wv = np.asarray(inputs["word_vectors"], np.float32)
    fc_w = np.asarray(inputs["fc_w"], np.float32)
    w1 = np.asarray(inputs["w1"], np.float32)
    w2 = np.asarray(inputs["w2"], np.float32)
    w3 = np.asarray(inputs["w3"], np.float32)
    w4 = np.asarray(inputs["w4"], np.float32)
    w5 = np.asarray(inputs["w5"], np.float32)
    T = np.asarray(inputs["T"], np.float32).reshape(1024, 4096)

    wvT = np.ascontiguousarray(wv.T)
    fcwT = np.ascontiguousarray(fc_w.T)
    w1T = np.ascontiguousarray(w1.transpose(2, 1, 3, 0))  # [kh, c, kw, cout]
    w2T = np.ascontiguousarray(w2.transpose(1, 2, 3, 0).reshape(128, 16, 256))
    w3T = np.ascontiguousarray(w3.transpose(1, 2, 3, 0).reshape(2, 128, 16, 256))
    w4T = np.ascontiguousarray(w4.transpose(1, 2, 3, 0).reshape(2, 128, 16, 4, 128).transpose(3, 0, 1, 2, 4))
    w5T = np.ascontiguousarray(w5.transpose(1, 2, 3, 0).reshape(4, 128, 16, 2, 128).transpose(3, 0, 1, 2, 4))

    def gbvec(g, b, n_t):
        out = np.zeros((128, 2 * n_t), np.float32)
        out[:, :n_t] = g.reshape(n_t, 128).T
        out[:, n_t:] = b.reshape(n_t, 128).T
        return out

    gb = np.zeros((128, 16), np.float32)
    gb[:, 0:4] = gbvec(np.asarray(inputs["g2"], np.float32), np.asarray(inputs["b2"], np.float32), 2)
    gb[:, 4:8] = gbvec(np.asarray(inputs["g3"], np.float32), np.asarray(inputs["b3"], np.float32), 2)
    gb[:, 8:16] = gbvec(np.asarray(inputs["g4"], np.float32), np.asarray(inputs["b4"], np.float32), 4)

    in_maps = []
    for r in range(R):
        in_maps.append({
            "image": np.ascontiguousarray(image[r * NS:(r + 1) * NS]),
            "wvT": wvT,
            "fcwT": np.ascontiguousarray(fcwT[:, r * 512:(r + 1) * 512]),
            "w1T": w1T, "w2T": w2T, "w3T": w3T, "w4T": w4T, "w5T": w5T,
            "Tsh": np.ascontiguousarray(T[:, r * 512:(r + 1) * 512].reshape(8, 128, 512)),
            "gb": gb,
        })
    return in_maps


def kernel(**inputs) -> np.ndarray:
    nc = _build()
    in_maps = _prep_in_maps(inputs)
    res = bass_utils.run_bass_kernel_spmd(nc, in_maps, core_ids=list(range(R)))
    feat = np.concatenate([res.results[r]["feat_out"] for r in range(R)], axis=0)
    mbd = np.concatenate([res.results[r]["mbd_out"] for r in range(R)], axis=1)
    return np.concatenate([feat, mbd], axis=1).astype(np.float32)


if __name__ == "__main__":
    _build()
    print("build ok")



# revision 4
# speedup vs baseline: 63.5681x; 1.2218x over previous
"""DiscriminatorStack on 8 trn2 cores.

Sharding: batch-parallel convs (16 img/core), hw-sharded fc + AllGather,
distributed BN stats via tiny AllReduce, bc-sharded minibatch discrimination
(AllGather of feat; each core computes all 128x128 pairs for its 32
mbd channels).

All matmul operands are bf16 (weights converted on host, activations
written as bf16 by the producing op); accumulation stays fp32 in PSUM,
BN statistics and the pairwise-exp path stay fp32.
"""
import sys
import numpy as np
import ml_dtypes

sys.path.insert(0, "/opt/trn_rl_repo")
sys.path.insert(0, "/opt/trn_rl_repo/concourse")

import concourse.bass as bass
import concourse.bacc as bacc
import concourse.mybir as mybir
import concourse.tile as tile
from concourse import bass_utils
from concourse.masks import make_identity

dt = mybir.dt
F32 = dt.float32
BF16 = dt.bfloat16
NPBF = ml_dtypes.bfloat16
AF = mybir.ActivationFunctionType
OP = mybir.AluOpType
AX = mybir.AxisListType

R = 8          # cores
NS = 16        # images per core
EPS = 1e-5

_CACHE = {}


def _lrelu_from_psum(nc, sb, psum_ap, out_ap, n_free, tagp):
    """out = lrelu(psum) via r1=relu(x), r2=relu(-x), out = r1 - 0.2*r2."""
    r1 = sb.tile([128, n_free], F32, tag="r1", bufs=2)
    r2 = sb.tile([128, n_free], F32, tag="r2", bufs=2)
    nc.scalar.activation(out=r1[:], in_=psum_ap, func=AF.Relu)
    nc.scalar.activation(out=r2[:], in_=psum_ap, func=AF.Relu, scale=-1.0)
    nc.vector.scalar_tensor_tensor(
        out=out_ap, in0=r2[:], scalar=-0.2, in1=r1[:], op0=OP.mult, op1=OP.add
    )


def _affine_lrelu(nc, sb, x_ap, out_ap, n_free, s_ap, t_ap, ns_ap, nt_ap, tagp):
    """out = lrelu(s*x + t) with per-partition scale/bias APs."""
    r1 = sb.tile([128, n_free], F32, tag="r1", bufs=2)
    r2 = sb.tile([128, n_free], F32, tag="r2", bufs=2)
    nc.scalar.activation(out=r1[:], in_=x_ap, func=AF.Relu, scale=s_ap, bias=t_ap)
    nc.scalar.activation(out=r2[:], in_=x_ap, func=AF.Relu, scale=ns_ap, bias=nt_ap)
    nc.vector.scalar_tensor_tensor(
        out=out_ap, in0=r2[:], scalar=-0.2, in1=r1[:], op0=OP.mult, op1=OP.add
    )


def _bn_finalize(nc, sb, dram, sums, sqs, nblk, n_ch_tiles, n_total, g_t, b_t, name):
    """sums/sqs: [128, n_ch_tiles*nblk] per-block partials. Returns
    (scale, bias, nscale, nbias) [128, n_ch_tiles] after AllReduce."""
    loc = sb.tile([128, 2 * n_ch_tiles], F32, tag=f"bnl{name}")
    nc.vector.tensor_reduce(
        out=loc[:, 0:n_ch_tiles],
        in_=sums[:].rearrange("p (c b) -> p c b", b=nblk),
        axis=AX.X, op=OP.add)
    nc.vector.tensor_reduce(
        out=loc[:, n_ch_tiles:2 * n_ch_tiles],
        in_=sqs[:].rearrange("p (c b) -> p c b", b=nblk),
        axis=AX.X, op=OP.add)
    bn_in = dram.tile([128, 2 * n_ch_tiles], F32)
    bn_out = dram.tile([128, 2 * n_ch_tiles], F32)
    nc.sync.dma_start(bn_in[:], loc[:])
    nc.gpsimd.collective_compute(
        "AllReduce", OP.add, replica_groups=[list(range(R))],
        ins=[bn_in.opt()], outs=[bn_out.opt()])
    glob = sb.tile([128, 2 * n_ch_tiles], F32, tag=f"bng{name}")
    nc.sync.dma_start(glob[:], bn_out[:])
    C = n_ch_tiles
    mean = sb.tile([128, C], F32, tag=f"bnm{name}")
    var = sb.tile([128, C], F32, tag=f"bnv{name}")
    inv = sb.tile([128, C], F32, tag=f"bni{name}")
    scale = sb.tile([128, C], F32, tag=f"bns{name}")
    bias = sb.tile([128, C], F32, tag=f"bnb{name}")
    nscale = sb.tile([128, C], F32, tag=f"bnns{name}")
    nbias = sb.tile([128, C], F32, tag=f"bnnb{name}")
    r = 1.0 / float(n_total)
    nc.vector.tensor_scalar(out=mean[:], in0=glob[:, 0:C], scalar1=r, scalar2=None, op0=OP.mult)
    nc.vector.tensor_scalar(out=var[:], in0=glob[:, C:2 * C], scalar1=r, scalar2=None, op0=OP.mult)
    # var = E[x^2] - mean^2  (in-place safe ops)
    m2 = sb.tile([128, C], F32, tag=f"bnm2{name}")
    nc.vector.tensor_tensor(out=m2[:], in0=mean[:], in1=mean[:], op=OP.mult)
    nc.vector.tensor_tensor(out=var[:], in0=var[:], in1=m2[:], op=OP.subtract)
    nc.vector.tensor_scalar(out=var[:], in0=var[:], scalar1=EPS, scalar2=None, op0=OP.add)
    # sqrt via ACT table + one Newton step (ACT tables are low precision)
    y0 = sb.tile([128, C], F32, tag=f"bny0{name}")
    yr = sb.tile([128, C], F32, tag=f"bnyr{name}")
    nc.scalar.activation(out=y0[:], in_=var[:], func=AF.Sqrt)
    nc.vector.reciprocal(yr[:], y0[:])
    nc.vector.tensor_tensor(out=yr[:], in0=var[:], in1=yr[:], op=OP.mult)
    nc.vector.tensor_tensor(out=y0[:], in0=y0[:], in1=yr[:], op=OP.add)
    nc.vector.tensor_scalar(out=y0[:], in0=y0[:], scalar1=0.5, scalar2=None, op0=OP.mult)
    nc.vector.reciprocal(inv[:], y0[:])
    nc.vector.tensor_tensor(out=scale[:], in0=g_t[:], in1=inv[:], op=OP.mult)
    nc.vector.tensor_tensor(out=bias[:], in0=mean[:], in1=scale[:], op=OP.mult)
    nc.vector.tensor_tensor(out=bias[:], in0=b_t[:], in1=bias[:], op=OP.subtract)
    nc.vector.tensor_scalar(out=nscale[:], in0=scale[:], scalar1=-1.0, scalar2=None, op0=OP.mult)
    nc.vector.tensor_scalar(out=nbias[:], in0=bias[:], scalar1=-1.0, scalar2=None, op0=OP.mult)
    return scale, bias, nscale, nbias


def _build():
    if "nc" in _CACHE:
        return _CACHE["nc"]
    nc = bacc.Bacc("TRN2", target_bir_lowering=False, debug=False, num_devices=R)

    image = nc.dram_tensor("image", [NS, 3, 64, 64], BF16, kind="ExternalInput")
    wvT = nc.dram_tensor("wvT", [4096, 128], BF16, kind="ExternalInput")
    fcwT = nc.dram_tensor("fcwT", [4096, 512], BF16, kind="ExternalInput")
    w1T = nc.dram_tensor("w1T", [4, 4, 4, 128], BF16, kind="ExternalInput")
    w2T = nc.dram_tensor("w2T", [128, 16, 256], BF16, kind="ExternalInput")
    w3T = nc.dram_tensor("w3T", [2, 128, 16, 256], BF16, kind="ExternalInput")
    w4T = nc.dram_tensor("w4T", [4, 2, 128, 16, 128], BF16, kind="ExternalInput")
    w5T = nc.dram_tensor("w5T", [2, 4, 128, 16, 128], BF16, kind="ExternalInput")
    Tsh = nc.dram_tensor("Tsh", [8, 128, 512], BF16, kind="ExternalInput")
    gb = nc.dram_tensor("gb", [128, 16], F32, kind="ExternalInput")
    # gb columns: g2(2) b2(2) g3(2) b3(2) g4(4) b4(4)

    feat_out = nc.dram_tensor("feat_out", [NS, 1024], F32, kind="ExternalOutput")
    mbd_out = nc.dram_tensor("mbd_out", [128, 32], F32, kind="ExternalOutput")

    with tile.TileContext(nc) as tc:
        with (
            tc.tile_pool(name="sb", bufs=1) as sb,
            tc.tile_pool(name="ps", bufs=1, space="PSUM") as ps,
            tc.tile_pool(name="dram", bufs=1, space="DRAM") as dram,
        ):
            ident = sb.tile([128, 128], F32, tag="ident")
            make_identity(nc, ident[:])
            identb = sb.tile([128, 128], BF16, tag="identb")
            nc.scalar.copy(identb[:], ident[:])
            nidentb = sb.tile([128, 128], BF16, tag="nidentb")
            nc.vector.tensor_scalar(out=nidentb[:], in0=ident[:], scalar1=-1.0,
                                    scalar2=None, op0=OP.mult)
            gbt = sb.tile([128, 16], F32, tag="gb")
            nc.sync.dma_start(gbt[:], gb.ap())

            # ---------------- fc ----------------
            wvT_sb = sb.tile([128, 32, 128], BF16, tag="wvT")
            nc.sync.dma_start(wvT_sb[:], wvT.ap().rearrange("(k p) n -> p k n", p=128))
            pf = ps.tile([128, 512], F32, tag="pb0")
            for kt in range(32):
                fcw_t = sb.tile([128, 512], BF16, tag="fcw", bufs=3)
                nc.sync.dma_start(fcw_t[:], fcwT.ap()[kt * 128:(kt + 1) * 128, :])
                nc.tensor.matmul(pf[:], wvT_sb[:, kt, :], fcw_t[:],
                                 start=(kt == 0), stop=(kt == 31))
            wv_sb = sb.tile([128, 512], BF16, tag="wvsb")
            nc.scalar.activation(out=wv_sb[:], in_=pf[:], func=AF.Relu)
            ag_fc_in = dram.tile([128, 512], BF16)
            ag_fc_out = dram.tile([8, 16, 512], BF16)
            nc.sync.dma_start(ag_fc_in[:], wv_sb[:])
            nc.gpsimd.collective_compute(
                "AllToAll", OP.bypass, replica_groups=[list(range(R))],
                ins=[ag_fc_in.opt()], outs=[ag_fc_out.opt()])

            # ---------------- conv1: K=16 (kh,c) contraction, kw accumulation ----------------
            a1_pad = sb.tile([128, 16, 34, 34], BF16, tag="bigact")
            nc.gpsimd.memset(a1_pad[:], 0.0)
            w1s = sb.tile([16, 4, 128], BF16, tag="w1")
            nc.sync.dma_start(w1s[:], w1T.ap().rearrange("kh c kw m -> (kh c) kw m"))
            agv = ag_fc_out[:].rearrange("r n (h w) -> r n h w", w=64)
            for pair in range(8):
                n0 = pair * 2
                # x0sh[kh*4+c, n, h', w] = xpad[c, n, h'+kh, w], h' = 0..63
                x0 = sb.tile([16, 2, 64, 66], BF16, tag="w3", name=f"x0_{pair}")
                nc.gpsimd.memset(x0[:], 0.0)
                wvs = sb.tile([1, 2, 64, 64], BF16, tag="actchain", name=f"wvs_{pair}")
                for n in range(2):
                    nc.sync.dma_start(
                        wvs[0:1, n].rearrange("p (a h) w -> p a (h w)", a=8),
                        agv[:, n0 + n].rearrange("a h w -> a (h w)")[None, :, :])
                for kh in range(4):
                    # padded rows h = h'+kh span image rows h-1 = h'+kh-1
                    hp0 = max(0, 1 - kh)            # first h' with valid image row
                    hpn = min(64, 65 - kh) - hp0    # h'+kh-1 <= 63 -> h' <= 64-kh
                    for n in range(2):
                        nc.sync.dma_start(
                            x0[4 * kh:4 * kh + 3, n, hp0:hp0 + hpn, 1:65],
                            image.ap()[n0 + n, 0:3, hp0 + kh - 1:hp0 + kh - 1 + hpn, :])
                        nc.sync.dma_start(
                            x0[4 * kh + 3:4 * kh + 4, n, hp0:hp0 + hpn, 1:65],
                            wvs[0:1, n, hp0 + kh - 1:hp0 + kh - 1 + hpn, :])
                for n in range(2):
                    for hh in range(2):
                        pc = ps.tile([128, 512], F32, tag=f"pb{(n * 2 + hh) % 4}",
                                     name=f"c1p{pair}_{n}_{hh}")
                        for kw in range(4):
                            rhs = x0[:, n, 2 * hh * 16:2 * hh * 16 + 31:2,
                                     kw:kw + 63:2]
                            nc.tensor.matmul(
                                pc[:], w1s[:, kw, :], rhs,
                                start=(kw == 0), stop=(kw == 3))
                        _lrelu_from_psum(
                            nc, sb, pc[:],
                            a1_pad[:, n0 + n, 1 + hh * 16:1 + (hh + 1) * 16, 1:33],
                            512, "c1")

            # ---------------- conv2 ----------------
            w2_sb = sb.tile([128, 16, 256], BF16, tag="w2")
            nc.sync.dma_start(w2_sb[:], w2T.ap())
            w3_sb = sb.tile([128, 2, 16, 256], BF16, tag="w3b")
            nc.sync.dma_start(w3_sb[:], w3T.ap().transpose([1, 0, 2, 3]))
            a2_raw = sb.tile([128, 2, 16, 16, 16], F32, tag="actchain")
            sums2 = sb.tile([128, 16], F32, tag="sums")
            sqs2 = sb.tile([128, 16], F32, tag="sqs")
            scr = sb.tile([128, 512], F32, tag="scr")
            for cot in range(2):
                pbl = [ps.tile([128, 512], F32, tag=f"pb{b}", name=f"c2p{cot}_{b}") for b in range(8)]
                for kk in range(16):
                    lhsT = w2_sb[:, kk, cot * 128:(cot + 1) * 128]
                    kh, kw = kk // 4, kk % 4
                    for blk in range(8):
                        rhs = a1_pad[:, 2 * blk:2 * blk + 2,
                                     kh:kh + 31:2, kw:kw + 31:2]
                        nc.tensor.matmul(pbl[blk][:], lhsT, rhs,
                                         start=(kk == 0), stop=(kk == 15))
                for blk in range(8):
                    nc.scalar.activation(
                        out=a2_raw[:, cot, 2 * blk:2 * blk + 2, :, :],
                        in_=pbl[blk][:], func=AF.Copy,
                        accum_out=sums2[:, cot * 8 + blk:cot * 8 + blk + 1])
                    nc.scalar.activation(
                        out=scr[:], in_=pbl[blk][:], func=AF.Square,
                        accum_out=sqs2[:, cot * 8 + blk:cot * 8 + blk + 1])
            s2, t2, ns2, nt2 = _bn_finalize(
                nc, sb, dram, sums2, sqs2, 8, 2, 128 * 256,
                gbt[:, 0:2], gbt[:, 2:4], "2")
            a2_pad = sb.tile([128, 2, 16, 18, 18], BF16, tag="bigact")
            nc.gpsimd.memset(a2_pad[:], 0.0)
            for cot in range(2):
                for i in range(16):
                    _affine_lrelu(
                        nc, sb, a2_raw[:, cot, i, :, :],
                        a2_pad[:, cot, i, 1:17, 1:17], 256,
                        s2[:, cot:cot + 1], t2[:, cot:cot + 1],
                        ns2[:, cot:cot + 1], nt2[:, cot:cot + 1], "n2")

            # ---------------- conv3 ----------------
            a3_raw = sb.tile([128, 2, 16, 8, 8], F32, tag="actchain")
            sums3 = sb.tile([128, 4], F32, tag="sums")
            sqs3 = sb.tile([128, 4], F32, tag="sqs")
            for cot in range(2):
                pbl = [ps.tile([128, 512], F32, tag=f"pb{b}", name=f"c3p{cot}_{b}") for b in range(2)]
                first = True
                for cit in range(2):
                    for kk in range(16):
                        lhsT = w3_sb[:, cit, kk, cot * 128:(cot + 1) * 128]
                        kh, kw = kk // 4, kk % 4
                        for blk in range(2):
                            rhs = a2_pad[:, cit, 8 * blk:8 * blk + 8,
                                         kh:kh + 15:2, kw:kw + 15:2]
                            nc.tensor.matmul(pbl[blk][:], lhsT, rhs,
                                             start=first,
                                             stop=(cit == 1 and kk == 15))
                        first = False
                for blk in range(2):
                    nc.scalar.activation(
                        out=a3_raw[:, cot, 8 * blk:8 * blk + 8, :, :],
                        in_=pbl[blk][:], func=AF.Copy,
                        accum_out=sums3[:, cot * 2 + blk:cot * 2 + blk + 1])
                    nc.scalar.activation(
                        out=scr[:], in_=pbl[blk][:], func=AF.Square,
                        accum_out=sqs3[:, cot * 2 + blk:cot * 2 + blk + 1])
            s3, t3, ns3, nt3 = _bn_finalize(
                nc, sb, dram, sums3, sqs3, 2, 2, 128 * 64,
                gbt[:, 4:6], gbt[:, 6:8], "3")
            a3_pad = sb.tile([128, 2, 16, 10, 10], BF16, tag="bigact")
            nc.gpsimd.memset(a3_pad[:], 0.0)
            for cot in range(2):
                for i in range(16):
                    _affine_lrelu(
                        nc, sb, a3_raw[:, cot, i, :, :],
                        a3_pad[:, cot, i, 1:9, 1:9], 64,
                        s3[:, cot:cot + 1], t3[:, cot:cot + 1],
                        ns3[:, cot:cot + 1], nt3[:, cot:cot + 1], "n3")

            # ---------------- conv4 ----------------
            a4_raw = sb.tile([128, 4, 16, 4, 4], F32, tag="actchain")
            sums4 = sb.tile([128, 4], F32, tag="sums")
            sqs4 = sb.tile([128, 4], F32, tag="sqs")
            for cot in range(4):
                w4c = sb.tile([128, 2, 16, 128], BF16, tag="w2")
                nc.sync.dma_start(
                    w4c[:], w4T.ap()[cot].transpose([1, 0, 2, 3]))
                pb = ps.tile([128, 256], F32, tag="pb0")
                first = True
                for cit in range(2):
                    for kk in range(16):
                        kh, kw = kk // 4, kk % 4
                        rhs = a3_pad[:, cit, :, kh:kh + 7:2, kw:kw + 7:2]
                        nc.tensor.matmul(pb[:], w4c[:, cit, kk, :], rhs,
                                         start=first,
                                         stop=(cit == 1 and kk == 15))
                        first = False
                nc.scalar.activation(
                    out=a4_raw[:, cot, :, :, :], in_=pb[:], func=AF.Copy,
                    accum_out=sums4[:, cot:cot + 1])
                nc.scalar.activation(
                    out=scr[:, 0:256], in_=pb[:], func=AF.Square,
                    accum_out=sqs4[:, cot:cot + 1])
            s4, t4, ns4, nt4 = _bn_finalize(
                nc, sb, dram, sums4, sqs4, 1, 4, 128 * 16,
                gbt[:, 8:12], gbt[:, 12:16], "4")
            a4_pad = sb.tile([128, 4, 16, 6, 6], BF16, tag="bigact")
            nc.gpsimd.memset(a4_pad[:], 0.0)
            for cot in range(4):
                for i in range(16):
                    _affine_lrelu(
                        nc, sb, a4_raw[:, cot, i, :, :],
                        a4_pad[:, cot, i, 1:5, 1:5], 16,
                        s4[:, cot:cot + 1], t4[:, cot:cot + 1],
                        ns4[:, cot:cot + 1], nt4[:, cot:cot + 1], "n4")

            # ---------------- conv5 -> feat ----------------
            a5_sb = sb.tile([128, 2, 16, 2, 2], F32, tag="a5")
            a5b = sb.tile([128, 2, 16, 2, 2], BF16, tag="a5b")
            for cot in range(2):
                w5c = sb.tile([128, 4, 16, 128], BF16, tag="w3")
                nc.sync.dma_start(
                    w5c[:], w5T.ap()[cot].transpose([1, 0, 2, 3]))
                pb = ps.tile([128, 64], F32, tag="pb1")
                first = True
                for cit in range(4):
                    for kk in range(16):
                        kh, kw = kk // 4, kk % 4
                        rhs = a4_pad[:, cit, :, kh:kh + 3:2, kw:kw + 3:2]
                        nc.tensor.matmul(pb[:], w5c[:, cit, kk, :], rhs,
                                         start=first,
                                         stop=(cit == 3 and kk == 15))
                        first = False
                nc.scalar.copy(a5_sb[:, cot, :, :, :], pb[:])
                nc.scalar.copy(a5b[:, cot, :, :, :], pb[:])
            ftl = dram.tile([16, 1024], BF16)
            ag_ft = dram.tile([8, 16, 1024], BF16)
            for ct in range(2):
                fo_dst = feat_out.ap().rearrange(
                    "n (ct c s) -> ct c n s", ct=2, s=4)[ct]
                src_ap = a5_sb[:, ct, :, :, :].rearrange("c n h w -> c n (h w)")
                nc.sync.dma_start(fo_dst, src_ap)
                fl_dst = ftl[:].rearrange("n (ct c s) -> ct c n s", ct=2, s=4)[ct]
                srcb_ap = a5b[:, ct, :, :, :].rearrange("c n h w -> c n (h w)")
                nc.sync.dma_start(fl_dst, srcb_ap)
            nc.gpsimd.collective_compute(
                "AllGather", OP.bypass, replica_groups=[list(range(R))],
                ins=[ftl.opt()], outs=[ag_ft.opt()])

            # ---------------- Ms = featT.T @ T_shard ----------------
            T_sb = sb.tile([128, 8, 512], BF16, tag="w3")
            nc.sync.dma_start(T_sb[:], Tsh.ap().transpose([1, 0, 2]))
            featn = sb.tile([128, 8, 128], BF16, tag="w2")
            nc.sync.dma_start(
                featn[:],
                ag_ft[:].rearrange("r n (at a) -> (r n) at a", at=8))
            fts = []
            for at in range(8):
                ptr = ps.tile([128, 128], BF16, tag=f"pb{at % 4}", name=f"ptr{at}")
                nc.tensor.transpose(ptr[:], featn[:, at, :], identb[:])
                ft = sb.tile([128, 128], BF16, tag=f"ft{at}", name=f"ftt{at}")
                nc.scalar.copy(ft[:], ptr[:])
                fts.append(ft)
            pms = ps.tile([128, 512], F32, tag="pb4")
            for at in range(8):
                nc.tensor.matmul(
                    pms[:], fts[at][:], T_sb[:, at, :],
                    start=(at == 0), stop=(at == 7))
            ms_p = sb.tile([128, 512], BF16, tag="msp")
            nc.scalar.copy(ms_p[:], pms[:])

            # ---------------- pairwise L1 + exp + sum ----------------
            exp_all = sb.tile([128, 32, 128], F32, tag="wvT")
            for jc in range(32):
                base = 4 * (jc % 2)
                dch = sb.tile([128, 4, 32], F32, tag="dch", bufs=2)
                for jj in range(4):
                    j = jc * 4 + jj
                    pbj = ps.tile([128, 512], F32, tag=f"pb{base + jj if base else jj}",
                                  name=f"pw{jc}_{jj}")
                    nc.tensor.matmul(
                        pbj[:], identb[:, j:j + 1].broadcast_to([128, 128]),
                        ms_p[:], start=True, stop=False)
                    nc.tensor.matmul(
                        pbj[:], nidentb[:], ms_p[:], start=False, stop=True)
                    nc.vector.tensor_reduce(
                        out=dch[:, jj, :],
                        in_=pbj[:].rearrange("p (b c) -> p b c", c=16),
                        axis=AX.X, op=OP.add, apply_absolute_value=True)
                nc.scalar.activation(
                    out=exp_all[:, :, jc * 4:(jc + 1) * 4].transpose([0, 2, 1]),
                    in_=dch[:], func=AF.Exp, scale=-1.0)
            mbd_sb = sb.tile([128, 32], F32, tag="mbd")
            nc.vector.tensor_reduce(
                out=mbd_sb[:], in_=exp_all[:], axis=AX.X, op=OP.add)
            nc.sync.dma_start(mbd_out.ap(), mbd_sb[:])

    nc.compile()
    _CACHE["nc"] = nc
    return nc


def _prep_in_maps(inputs):
    image = np.asarray(inputs["image"], np.float32)
    wv = np.asarray(inputs["word_vectors"], np.float32)
    fc_w = np.asarray(inputs["fc_w"], np.float32)
    w1 = np.asarray(inputs["w1"], np.float32)
    w2 = np.asarray(inputs["w2"], np.float32)
    w3 = np.asarray(inputs["w3"], np.float32)
    w4 = np.asarray(inputs["w4"], np.float32)
    w5 = np.asarray(inputs["w5"], np.float32)
    T = np.asarray(inputs["T"], np.float32).reshape(1024, 4096)

    wvT = np.ascontiguousarray(wv.T).astype(NPBF)
    fcwT = np.ascontiguousarray(fc_w.T).astype(NPBF)
    w1T = np.ascontiguousarray(w1.transpose(2, 1, 3, 0)).astype(NPBF)  # [kh, c, kw, cout]
    w2T = np.ascontiguousarray(w2.transpose(1, 2, 3, 0).reshape(128, 16, 256)).astype(NPBF)
    w3T = np.ascontiguousarray(w3.transpose(1, 2, 3, 0).reshape(2, 128, 16, 256)).astype(NPBF)
    w4T = np.ascontiguousarray(w4.transpose(1, 2, 3, 0).reshape(2, 128, 16, 4, 128).transpose(3, 0, 1, 2, 4)).astype(NPBF)
    w5T = np.ascontiguousarray(w5.transpose(1, 2, 3, 0).reshape(4, 128, 16, 2, 128).transpose(3, 0, 1, 2, 4)).astype(NPBF)
    image_b = image.astype(NPBF)

    def gbvec(g, b, n_t):
        out = np.zeros((128, 2 * n_t), np.float32)
        out[:, :n_t] = g.reshape(n_t, 128).T
        out[:, n_t:] = b.reshape(n_t, 128).T
        return out

    gb = np.zeros((128, 16), np.float32)
    gb[:, 0:4] = gbvec(np.asarray(inputs["g2"], np.float32), np.asarray(inputs["b2"], np.float32), 2)
    gb[:, 4:8] = gbvec(np.asarray(inputs["g3"], np.float32), np.asarray(inputs["b3"], np.float32), 2)
    gb[:, 8:16] = gbvec(np.asarray(inputs["g4"], np.float32), np.asarray(inputs["b4"], np.float32), 4)

    in_maps = []
    for r in range(R):
        in_maps.append({
            "image": np.ascontiguousarray(image_b[r * NS:(r + 1) * NS]),
            "wvT": wvT,
            "fcwT": np.ascontiguousarray(fcwT[:, r * 512:(r + 1) * 512]),
            "w1T": w1T, "w2T": w2T, "w3T": w3T, "w4T": w4T, "w5T": w5T,
            "Tsh": np.ascontiguousarray(
                T[:, r * 512:(r + 1) * 512].reshape(8, 128, 512)).astype(NPBF),
            "gb": gb,
        })
    return in_maps


def kernel(**inputs) -> np.ndarray:
    nc = _build()
    in_maps = _prep_in_maps(inputs)
    res = bass_utils.run_bass_kernel_spmd(nc, in_maps, core_ids=list(range(R)))
    feat = np.concatenate([res.results[r]["feat_out"] for r in range(R)], axis=0)
    mbd = np.concatenate([res.results[r]["mbd_out"] for r in range(R)], axis=1)
    return np.concatenate([feat, mbd], axis=1).astype(np.float32)


if __name__ == "__main__":
    _build()
    print("build ok")


# revision 5
# speedup vs baseline: 130.8526x; 2.0585x over previous
"""DiscriminatorStack on 8 trn2 cores.

Sharding: batch-parallel convs (16 img/core), hw-sharded fc + AllGather,
distributed BN stats via tiny AllReduce, bc-sharded minibatch discrimination
(AllGather of feat; each core computes all 128x128 pairs for its 32
mbd channels).

All matmul operands are bf16 (weights converted on host, activations
written as bf16 by the producing op); accumulation stays fp32 in PSUM,
BN statistics and the pairwise-exp path stay fp32.
"""
import sys
import numpy as np
import ml_dtypes

sys.path.insert(0, "/opt/trn_rl_repo")
sys.path.insert(0, "/opt/trn_rl_repo/concourse")

import concourse.bass as bass
import concourse.bacc as bacc
import concourse.mybir as mybir
import concourse.tile as tile
from concourse import bass_utils
from concourse.masks import make_identity

dt = mybir.dt
F32 = dt.float32
BF16 = dt.bfloat16
NPBF = ml_dtypes.bfloat16
AF = mybir.ActivationFunctionType
OP = mybir.AluOpType
AX = mybir.AxisListType

R = 8          # cores
NS = 16        # images per core
EPS = 1e-5

_CACHE = {}

# packed-blob element offsets (bf16 elements)
_SIZES = [
    ("image", 16 * 3 * 64 * 64),
    ("wvT", 4096 * 128),
    ("fcwT", 4096 * 512),
    ("w1T", 4 * 4 * 4 * 128),
    ("w2T", 128 * 16 * 256),
    ("w3T", 2 * 128 * 16 * 256),
    ("w4T", 4 * 2 * 128 * 16 * 128),
    ("w5T", 2 * 4 * 128 * 16 * 128),
    ("Tsh", 8 * 128 * 512),
    ("gb", 128 * 32),
]
OFF = {}
_o = 0
for _n, _s in _SIZES:
    OFF[_n] = _o
    _o += _s
BLOB_N = _o


def _lrelu_from_psum(nc, sb, psum_ap, out_ap, n_free, tagp):
    """out = lrelu(psum) via r1=relu(x), r2=relu(-x), out = r1 - 0.2*r2."""
    r1 = sb.tile([128, n_free], F32, tag="r1", bufs=2)
    r2 = sb.tile([128, n_free], F32, tag="r2", bufs=2)
    nc.scalar.activation(out=r1[:], in_=psum_ap, func=AF.Relu)
    nc.scalar.activation(out=r2[:], in_=psum_ap, func=AF.Relu, scale=-1.0)
    nc.vector.scalar_tensor_tensor(
        out=out_ap, in0=r2[:], scalar=-0.2, in1=r1[:], op0=OP.mult, op1=OP.add
    )


def _affine_lrelu(nc, sb, x_ap, out_ap, n_free, s_ap, t_ap, ns_ap, nt_ap, tagp):
    """out = lrelu(s*x + t) with per-partition scale/bias APs."""
    r1 = sb.tile([128, n_free], F32, tag="r1", bufs=2)
    r2 = sb.tile([128, n_free], F32, tag="r2", bufs=2)
    nc.scalar.activation(out=r1[:], in_=x_ap, func=AF.Relu, scale=s_ap, bias=t_ap)
    nc.scalar.activation(out=r2[:], in_=x_ap, func=AF.Relu, scale=ns_ap, bias=nt_ap)
    nc.vector.scalar_tensor_tensor(
        out=out_ap, in0=r2[:], scalar=-0.2, in1=r1[:], op0=OP.mult, op1=OP.add
    )


def _bn_finalize(nc, sb, dram, sums, sqs, nblk, n_ch_tiles, n_total, g_t, b_t, name):
    """sums/sqs: [128, n_ch_tiles*nblk] per-block partials. Returns
    (scale, bias, nscale, nbias) [128, n_ch_tiles] after AllReduce."""
    loc = sb.tile([128, 2 * n_ch_tiles], F32, tag=f"bnl{name}")
    nc.vector.tensor_reduce(
        out=loc[:, 0:n_ch_tiles],
        in_=sums[:].rearrange("p (c b) -> p c b", b=nblk),
        axis=AX.X, op=OP.add)
    nc.vector.tensor_reduce(
        out=loc[:, n_ch_tiles:2 * n_ch_tiles],
        in_=sqs[:].rearrange("p (c b) -> p c b", b=nblk),
        axis=AX.X, op=OP.add)
    bn_in = dram.tile([128, 2 * n_ch_tiles], F32)
    bn_out = dram.tile([128, 2 * n_ch_tiles], F32)
    nc.sync.dma_start(bn_in[:], loc[:])
    nc.gpsimd.collective_compute(
        "AllReduce", OP.add, replica_groups=[list(range(R))],
        ins=[bn_in.opt()], outs=[bn_out.opt()])
    glob = sb.tile([128, 2 * n_ch_tiles], F32, tag=f"bng{name}")
    nc.sync.dma_start(glob[:], bn_out[:])
    C = n_ch_tiles
    mean = sb.tile([128, C], F32, tag=f"bnm{name}")
    var = sb.tile([128, C], F32, tag=f"bnv{name}")
    inv = sb.tile([128, C], F32, tag=f"bni{name}")
    scale = sb.tile([128, C], F32, tag=f"bns{name}")
    bias = sb.tile([128, C], F32, tag=f"bnb{name}")
    nscale = sb.tile([128, C], F32, tag=f"bnns{name}")
    nbias = sb.tile([128, C], F32, tag=f"bnnb{name}")
    r = 1.0 / float(n_total)
    nc.vector.tensor_scalar(out=mean[:], in0=glob[:, 0:C], scalar1=r, scalar2=None, op0=OP.mult)
    nc.vector.tensor_scalar(out=var[:], in0=glob[:, C:2 * C], scalar1=r, scalar2=None, op0=OP.mult)
    # var = E[x^2] - mean^2  (in-place safe ops)
    m2 = sb.tile([128, C], F32, tag=f"bnm2{name}")
    nc.vector.tensor_tensor(out=m2[:], in0=mean[:], in1=mean[:], op=OP.mult)
    nc.vector.tensor_tensor(out=var[:], in0=var[:], in1=m2[:], op=OP.subtract)
    nc.vector.tensor_scalar(out=var[:], in0=var[:], scalar1=EPS, scalar2=None, op0=OP.add)
    # sqrt via ACT table + one Newton step (ACT tables are low precision)
    y0 = sb.tile([128, C], F32, tag=f"bny0{name}")
    yr = sb.tile([128, C], F32, tag=f"bnyr{name}")
    nc.scalar.activation(out=y0[:], in_=var[:], func=AF.Sqrt)
    nc.vector.reciprocal(yr[:], y0[:])
    nc.vector.tensor_tensor(out=yr[:], in0=var[:], in1=yr[:], op=OP.mult)
    nc.vector.tensor_tensor(out=y0[:], in0=y0[:], in1=yr[:], op=OP.add)
    nc.vector.tensor_scalar(out=y0[:], in0=y0[:], scalar1=0.5, scalar2=None, op0=OP.mult)
    nc.vector.reciprocal(inv[:], y0[:])
    nc.vector.tensor_tensor(out=scale[:], in0=g_t[:], in1=inv[:], op=OP.mult)
    nc.vector.tensor_tensor(out=bias[:], in0=mean[:], in1=scale[:], op=OP.mult)
    nc.vector.tensor_tensor(out=bias[:], in0=b_t[:], in1=bias[:], op=OP.subtract)
    nc.vector.tensor_scalar(out=nscale[:], in0=scale[:], scalar1=-1.0, scalar2=None, op0=OP.mult)
    nc.vector.tensor_scalar(out=nbias[:], in0=bias[:], scalar1=-1.0, scalar2=None, op0=OP.mult)
    return scale, bias, nscale, nbias


def _build():
    if "nc" in _CACHE:
        return _CACHE["nc"]
    nc = bacc.Bacc("TRN2", target_bir_lowering=False, debug=False, num_devices=R)

    # single packed input: all bf16 tensors + gb (fp32 bits viewed as bf16)
    blob = nc.dram_tensor("blob", [BLOB_N], BF16, kind="ExternalInput")
    bv = blob.ap()
    image = bv[OFF["image"]:OFF["image"] + 196608].rearrange(
        "(n c h w) -> n c h w", c=3, h=64, w=64)
    wvT_v = bv[OFF["wvT"]:OFF["wvT"] + 524288].rearrange(
        "(k p n) -> p k n", p=128, n=128)
    fcwT_v = bv[OFF["fcwT"]:OFF["fcwT"] + 2097152].rearrange(
        "(k n) -> k n", n=512)
    w1T_v = bv[OFF["w1T"]:OFF["w1T"] + 8192].rearrange(
        "(a kw m) -> a kw m", kw=4, m=128)
    w2T_v = bv[OFF["w2T"]:OFF["w2T"] + 524288].rearrange(
        "(p k m) -> p k m", k=16, m=256)
    w3T_v = bv[OFF["w3T"]:OFF["w3T"] + 1048576].rearrange(
        "(c p k m) -> c p k m", p=128, k=16, m=256)
    w4T_v = bv[OFF["w4T"]:OFF["w4T"] + 2097152].rearrange(
        "(t c p k m) -> t c p k m", c=2, p=128, k=16, m=128)
    w5T_v = bv[OFF["w5T"]:OFF["w5T"] + 2097152].rearrange(
        "(t c p k m) -> t c p k m", c=4, p=128, k=16, m=128)
    Tsh_v = bv[OFF["Tsh"]:OFF["Tsh"] + 524288].rearrange(
        "(a p n) -> a p n", p=128, n=512)
    gb_v = bv[OFF["gb"]:OFF["gb"] + 4096].rearrange("(p f) -> p f", f=32)
    # gb columns (after fp32 bitcast): g2(2) b2(2) g3(2) b3(2) g4(4) b4(4)

    out = nc.dram_tensor("out", [NS * 1024 + 128 * 32], F32, kind="ExternalOutput")
    feat_dst = out.ap()[0:NS * 1024].rearrange(
        "(n ct c s) -> ct c n s", ct=2, c=128, s=4)
    mbd_dst = out.ap()[NS * 1024:NS * 1024 + 4096].rearrange("(p f) -> p f", f=32)

    with tile.TileContext(nc) as tc:
        with (
            tc.tile_pool(name="sb", bufs=1) as sb,
            tc.tile_pool(name="ps", bufs=1, space="PSUM") as ps,
            tc.tile_pool(name="dram", bufs=1, space="DRAM") as dram,
        ):
            ident = sb.tile([128, 128], F32, tag="ident")
            make_identity(nc, ident[:])
            identb = sb.tile([128, 128], BF16, tag="identb")
            nc.scalar.copy(identb[:], ident[:])
            nidentb = sb.tile([128, 128], BF16, tag="nidentb")
            nc.vector.tensor_scalar(out=nidentb[:], in0=ident[:], scalar1=-1.0,
                                    scalar2=None, op0=OP.mult)
            gbt2 = sb.tile([128, 32], BF16, tag="gb")
            nc.sync.dma_start(gbt2[:], gb_v)
            gbt = gbt2[:].bitcast(F32)

            # ---------------- fc ----------------
            wvT_sb = sb.tile([128, 32, 128], BF16, tag="wvT")
            nc.sync.dma_start(wvT_sb[:], wvT_v)
            pf = ps.tile([128, 512], F32, tag="pb0")
            for kt in range(32):
                fcw_t = sb.tile([128, 512], BF16, tag="fcw", bufs=3)
                nc.sync.dma_start(fcw_t[:], fcwT_v[kt * 128:(kt + 1) * 128, :])
                nc.tensor.matmul(pf[:], wvT_sb[:, kt, :], fcw_t[:],
                                 start=(kt == 0), stop=(kt == 31))
            wv_sb = sb.tile([128, 512], BF16, tag="wvsb")
            nc.scalar.activation(out=wv_sb[:], in_=pf[:], func=AF.Relu)
            ag_fc_in = dram.tile([128, 512], BF16)
            ag_fc_out = dram.tile([8, 16, 512], BF16)
            nc.sync.dma_start(ag_fc_in[:], wv_sb[:])
            nc.gpsimd.collective_compute(
                "AllToAll", OP.bypass, replica_groups=[list(range(R))],
                ins=[ag_fc_in.opt()], outs=[ag_fc_out.opt()])

            # ---------------- conv1: K=16 (kh,c) contraction, kw accumulation ----------------
            a1_pad = sb.tile([128, 16, 34, 34], BF16, tag="bigact")
            nc.gpsimd.memset(a1_pad[:], 0.0)
            w1s = sb.tile([16, 4, 128], BF16, tag="w1")
            nc.sync.dma_start(w1s[:], w1T_v)
            agv = ag_fc_out[:].rearrange("r n (h w) -> r n h w", w=64)
            for pair in range(8):
                n0 = pair * 2
                # x0sh[kh*4+c, n, h', w] = xpad[c, n, h'+kh, w], h' = 0..63
                x0 = sb.tile([16, 2, 64, 66], BF16, tag="w3", name=f"x0_{pair}")
                nc.gpsimd.memset(x0[:], 0.0)
                wvs = sb.tile([1, 2, 64, 64], BF16, tag="actchain", name=f"wvs_{pair}")
                for n in range(2):
                    nc.sync.dma_start(
                        wvs[0:1, n].rearrange("p (a h) w -> p a (h w)", a=8),
                        agv[:, n0 + n].rearrange("a h w -> a (h w)")[None, :, :])
                for kh in range(4):
                    # padded rows h = h'+kh span image rows h-1 = h'+kh-1
                    hp0 = max(0, 1 - kh)            # first h' with valid image row
                    hpn = min(64, 65 - kh) - hp0    # h'+kh-1 <= 63 -> h' <= 64-kh
                    for n in range(2):
                        nc.sync.dma_start(
                            x0[4 * kh:4 * kh + 3, n, hp0:hp0 + hpn, 1:65],
                            image[n0 + n, 0:3, hp0 + kh - 1:hp0 + kh - 1 + hpn, :])
                        nc.sync.dma_start(
                            x0[4 * kh + 3:4 * kh + 4, n, hp0:hp0 + hpn, 1:65],
                            wvs[0:1, n, hp0 + kh - 1:hp0 + kh - 1 + hpn, :])
                for n in range(2):
                    for hh in range(2):
                        pc = ps.tile([128, 512], F32, tag=f"pb{(n * 2 + hh) % 4}",
                                     name=f"c1p{pair}_{n}_{hh}")
                        for kw in range(4):
                            rhs = x0[:, n, 2 * hh * 16:2 * hh * 16 + 31:2,
                                     kw:kw + 63:2]
                            nc.tensor.matmul(
                                pc[:], w1s[:, kw, :], rhs,
                                start=(kw == 0), stop=(kw == 3))
                        _lrelu_from_psum(
                            nc, sb, pc[:],
                            a1_pad[:, n0 + n, 1 + hh * 16:1 + (hh + 1) * 16, 1:33],
                            512, "c1")

            # ---------------- conv2 ----------------
            w2_sb = sb.tile([128, 16, 256], BF16, tag="w2")
            nc.sync.dma_start(w2_sb[:], w2T_v)
            w3_sb = sb.tile([128, 2, 16, 256], BF16, tag="w3b")
            nc.sync.dma_start(w3_sb[:], w3T_v.transpose([1, 0, 2, 3]))
            a2_raw = sb.tile([128, 2, 16, 16, 16], F32, tag="actchain")
            sums2 = sb.tile([128, 16], F32, tag="sums")
            sqs2 = sb.tile([128, 16], F32, tag="sqs")
            scr = sb.tile([128, 512], F32, tag="scr")
            for cot in range(2):
                pbl = [ps.tile([128, 512], F32, tag=f"pb{b}", name=f"c2p{cot}_{b}") for b in range(8)]
                for kk in range(16):
                    lhsT = w2_sb[:, kk, cot * 128:(cot + 1) * 128]
                    kh, kw = kk // 4, kk % 4
                    for blk in range(8):
                        rhs = a1_pad[:, 2 * blk:2 * blk + 2,
                                     kh:kh + 31:2, kw:kw + 31:2]
                        nc.tensor.matmul(pbl[blk][:], lhsT, rhs,
                                         start=(kk == 0), stop=(kk == 15))
                for blk in range(8):
                    nc.scalar.activation(
                        out=a2_raw[:, cot, 2 * blk:2 * blk + 2, :, :],
                        in_=pbl[blk][:], func=AF.Copy,
                        accum_out=sums2[:, cot * 8 + blk:cot * 8 + blk + 1])
                    nc.scalar.activation(
                        out=scr[:], in_=pbl[blk][:], func=AF.Square,
                        accum_out=sqs2[:, cot * 8 + blk:cot * 8 + blk + 1])
            s2, t2, ns2, nt2 = _bn_finalize(
                nc, sb, dram, sums2, sqs2, 8, 2, 128 * 256,
                gbt[:, 0:2], gbt[:, 2:4], "2")
            a2_pad = sb.tile([128, 2, 16, 18, 18], BF16, tag="bigact")
            nc.gpsimd.memset(a2_pad[:], 0.0)
            for cot in range(2):
                for i in range(16):
                    _affine_lrelu(
                        nc, sb, a2_raw[:, cot, i, :, :],
                        a2_pad[:, cot, i, 1:17, 1:17], 256,
                        s2[:, cot:cot + 1], t2[:, cot:cot + 1],
                        ns2[:, cot:cot + 1], nt2[:, cot:cot + 1], "n2")

            # ---------------- conv3 ----------------
            a3_raw = sb.tile([128, 2, 16, 8, 8], F32, tag="actchain")
            sums3 = sb.tile([128, 4], F32, tag="sums")
            sqs3 = sb.tile([128, 4], F32, tag="sqs")
            for cot in range(2):
                pbl = [ps.tile([128, 512], F32, tag=f"pb{b}", name=f"c3p{cot}_{b}") for b in range(2)]
                first = True
                for cit in range(2):
                    for kk in range(16):
                        lhsT = w3_sb[:, cit, kk, cot * 128:(cot + 1) * 128]
                        kh, kw = kk // 4, kk % 4
                        for blk in range(2):
                            rhs = a2_pad[:, cit, 8 * blk:8 * blk + 8,
                                         kh:kh + 15:2, kw:kw + 15:2]
                            nc.tensor.matmul(pbl[blk][:], lhsT, rhs,
                                             start=first,
                                             stop=(cit == 1 and kk == 15))
                        first = False
                for blk in range(2):
                    nc.scalar.activation(
                        out=a3_raw[:, cot, 8 * blk:8 * blk + 8, :, :],
                        in_=pbl[blk][:], func=AF.Copy,
                        accum_out=sums3[:, cot * 2 + blk:cot * 2 + blk + 1])
                    nc.scalar.activation(
                        out=scr[:], in_=pbl[blk][:], func=AF.Square,
                        accum_out=sqs3[:, cot * 2 + blk:cot * 2 + blk + 1])
            s3, t3, ns3, nt3 = _bn_finalize(
                nc, sb, dram, sums3, sqs3, 2, 2, 128 * 64,
                gbt[:, 4:6], gbt[:, 6:8], "3")
            a3_pad = sb.tile([128, 2, 16, 10, 10], BF16, tag="bigact")
            nc.gpsimd.memset(a3_pad[:], 0.0)
            for cot in range(2):
                for i in range(16):
                    _affine_lrelu(
                        nc, sb, a3_raw[:, cot, i, :, :],
                        a3_pad[:, cot, i, 1:9, 1:9], 64,
                        s3[:, cot:cot + 1], t3[:, cot:cot + 1],
                        ns3[:, cot:cot + 1], nt3[:, cot:cot + 1], "n3")

            # ---------------- conv4 ----------------
            a4_raw = sb.tile([128, 4, 16, 4, 4], F32, tag="actchain")
            sums4 = sb.tile([128, 4], F32, tag="sums")
            sqs4 = sb.tile([128, 4], F32, tag="sqs")
            for cot in range(4):
                w4c = sb.tile([128, 2, 16, 128], BF16, tag="w2")
                nc.sync.dma_start(
                    w4c[:], w4T_v[cot].transpose([1, 0, 2, 3]))
                pb = ps.tile([128, 256], F32, tag="pb0")
                first = True
                for cit in range(2):
                    for kk in range(16):
                        kh, kw = kk // 4, kk % 4
                        rhs = a3_pad[:, cit, :, kh:kh + 7:2, kw:kw + 7:2]
                        nc.tensor.matmul(pb[:], w4c[:, cit, kk, :], rhs,
                                         start=first,
                                         stop=(cit == 1 and kk == 15))
                        first = False
                nc.scalar.activation(
                    out=a4_raw[:, cot, :, :, :], in_=pb[:], func=AF.Copy,
                    accum_out=sums4[:, cot:cot + 1])
                nc.scalar.activation(
                    out=scr[:, 0:256], in_=pb[:], func=AF.Square,
                    accum_out=sqs4[:, cot:cot + 1])
            s4, t4, ns4, nt4 = _bn_finalize(
                nc, sb, dram, sums4, sqs4, 1, 4, 128 * 16,
                gbt[:, 8:12], gbt[:, 12:16], "4")
            a4_pad = sb.tile([128, 4, 16, 6, 6], BF16, tag="bigact")
            nc.gpsimd.memset(a4_pad[:], 0.0)
            for cot in range(4):
                for i in range(16):
                    _affine_lrelu(
                        nc, sb, a4_raw[:, cot, i, :, :],
                        a4_pad[:, cot, i, 1:5, 1:5], 16,
                        s4[:, cot:cot + 1], t4[:, cot:cot + 1],
                        ns4[:, cot:cot + 1], nt4[:, cot:cot + 1], "n4")

            # ---------------- conv5 -> feat ----------------
            a5_sb = sb.tile([128, 2, 16, 2, 2], F32, tag="a5")
            a5b = sb.tile([128, 2, 16, 2, 2], BF16, tag="a5b")
            for cot in range(2):
                w5c = sb.tile([128, 4, 16, 128], BF16, tag="w3")
                nc.sync.dma_start(
                    w5c[:], w5T_v[cot].transpose([1, 0, 2, 3]))
                pb = ps.tile([128, 64], F32, tag="pb1")
                first = True
                for cit in range(4):
                    for kk in range(16):
                        kh, kw = kk // 4, kk % 4
                        rhs = a4_pad[:, cit, :, kh:kh + 3:2, kw:kw + 3:2]
                        nc.tensor.matmul(pb[:], w5c[:, cit, kk, :], rhs,
                                         start=first,
                                         stop=(cit == 3 and kk == 15))
                        first = False
                nc.scalar.copy(a5_sb[:, cot, :, :, :], pb[:])
                nc.scalar.copy(a5b[:, cot, :, :, :], pb[:])
            ftl = dram.tile([16, 1024], BF16)
            ag_ft = dram.tile([8, 16, 1024], BF16)
            for ct in range(2):
                fo_dst = feat_dst[ct]
                src_ap = a5_sb[:, ct, :, :, :].rearrange("c n h w -> c n (h w)")
                nc.sync.dma_start(fo_dst, src_ap)
                fl_dst = ftl[:].rearrange("n (ct c s) -> ct c n s", ct=2, s=4)[ct]
                srcb_ap = a5b[:, ct, :, :, :].rearrange("c n h w -> c n (h w)")
                nc.sync.dma_start(fl_dst, srcb_ap)
            nc.gpsimd.collective_compute(
                "AllGather", OP.bypass, replica_groups=[list(range(R))],
                ins=[ftl.opt()], outs=[ag_ft.opt()])

            # ---------------- Ms = featT.T @ T_shard ----------------
            T_sb = sb.tile([128, 8, 512], BF16, tag="w3")
            nc.sync.dma_start(T_sb[:], Tsh_v.transpose([1, 0, 2]))
            featn = sb.tile([128, 8, 128], BF16, tag="w2")
            nc.sync.dma_start(
                featn[:],
                ag_ft[:].rearrange("r n (at a) -> (r n) at a", at=8))
            fts = []
            for at in range(8):
                ptr = ps.tile([128, 128], BF16, tag=f"pb{at % 4}", name=f"ptr{at}")
                nc.tensor.transpose(ptr[:], featn[:, at, :], identb[:])
                ft = sb.tile([128, 128], BF16, tag=f"ft{at}", name=f"ftt{at}")
                nc.scalar.copy(ft[:], ptr[:])
                fts.append(ft)
            pms = ps.tile([128, 512], F32, tag="pb4")
            for at in range(8):
                nc.tensor.matmul(
                    pms[:], fts[at][:], T_sb[:, at, :],
                    start=(at == 0), stop=(at == 7))
            ms_p = sb.tile([128, 512], BF16, tag="msp")
            nc.scalar.copy(ms_p[:], pms[:])

            # ---------------- pairwise L1 + exp + sum ----------------
            exp_all = sb.tile([128, 32, 128], F32, tag="wvT")
            for jc in range(32):
                base = 4 * (jc % 2)
                dch = sb.tile([128, 4, 32], F32, tag="dch", bufs=2)
                for jj in range(4):
                    j = jc * 4 + jj
                    pbj = ps.tile([128, 512], F32, tag=f"pb{base + jj if base else jj}",
                                  name=f"pw{jc}_{jj}")
                    nc.tensor.matmul(
                        pbj[:], identb[:, j:j + 1].broadcast_to([128, 128]),
                        ms_p[:], start=True, stop=False)
                    nc.tensor.matmul(
                        pbj[:], nidentb[:], ms_p[:], start=False, stop=True)
                    nc.vector.tensor_reduce(
                        out=dch[:, jj, :],
                        in_=pbj[:].rearrange("p (b c) -> p b c", c=16),
                        axis=AX.X, op=OP.add, apply_absolute_value=True)
                nc.scalar.activation(
                    out=exp_all[:, :, jc * 4:(jc + 1) * 4].transpose([0, 2, 1]),
                    in_=dch[:], func=AF.Exp, scale=-1.0)
            mbd_sb = sb.tile([128, 32], F32, tag="mbd")
            nc.vector.tensor_reduce(
                out=mbd_sb[:], in_=exp_all[:], axis=AX.X, op=OP.add)
            nc.sync.dma_start(mbd_dst, mbd_sb[:])

    nc.compile()
    _CACHE["nc"] = nc
    return nc


def _prep_in_maps(inputs):
    image = np.asarray(inputs["image"], np.float32)
    wv = np.asarray(inputs["word_vectors"], np.float32)
    fc_w = np.asarray(inputs["fc_w"], np.float32)
    w1 = np.asarray(inputs["w1"], np.float32)
    w2 = np.asarray(inputs["w2"], np.float32)
    w3 = np.asarray(inputs["w3"], np.float32)
    w4 = np.asarray(inputs["w4"], np.float32)
    w5 = np.asarray(inputs["w5"], np.float32)
    T = np.asarray(inputs["T"], np.float32).reshape(1024, 4096)

    wvT = np.ascontiguousarray(wv.T).astype(NPBF)
    fcwT = np.ascontiguousarray(fc_w.T).astype(NPBF)
    w1T = np.ascontiguousarray(w1.transpose(2, 1, 3, 0)).astype(NPBF)  # [kh, c, kw, cout]
    w2T = np.ascontiguousarray(w2.transpose(1, 2, 3, 0).reshape(128, 16, 256)).astype(NPBF)
    w3T = np.ascontiguousarray(w3.transpose(1, 2, 3, 0).reshape(2, 128, 16, 256)).astype(NPBF)
    w4T = np.ascontiguousarray(w4.transpose(1, 2, 3, 0).reshape(2, 128, 16, 4, 128).transpose(3, 0, 1, 2, 4)).astype(NPBF)
    w5T = np.ascontiguousarray(w5.transpose(1, 2, 3, 0).reshape(4, 128, 16, 2, 128).transpose(3, 0, 1, 2, 4)).astype(NPBF)
    image_b = image.astype(NPBF)

    def gbvec(g, b, n_t):
        out = np.zeros((128, 2 * n_t), np.float32)
        out[:, :n_t] = g.reshape(n_t, 128).T
        out[:, n_t:] = b.reshape(n_t, 128).T
        return out

    gb = np.zeros((128, 16), np.float32)
    gb[:, 0:4] = gbvec(np.asarray(inputs["g2"], np.float32), np.asarray(inputs["b2"], np.float32), 2)
    gb[:, 4:8] = gbvec(np.asarray(inputs["g3"], np.float32), np.asarray(inputs["b3"], np.float32), 2)
    gb[:, 8:16] = gbvec(np.asarray(inputs["g4"], np.float32), np.asarray(inputs["b4"], np.float32), 4)

    gb_bits = np.ascontiguousarray(gb).view(NPBF)  # [128, 32] raw fp32 bits

    in_maps = []
    for r in range(R):
        parts = [
            np.ascontiguousarray(image_b[r * NS:(r + 1) * NS]).ravel(),
            wvT.ravel(),
            np.ascontiguousarray(fcwT[:, r * 512:(r + 1) * 512]).ravel(),
            w1T.ravel(), w2T.ravel(), w3T.ravel(), w4T.ravel(), w5T.ravel(),
            np.ascontiguousarray(
                T[:, r * 512:(r + 1) * 512].reshape(8, 128, 512)).astype(NPBF).ravel(),
            gb_bits.ravel(),
        ]
        blob = np.concatenate(parts)
        assert blob.shape[0] == BLOB_N, (blob.shape, BLOB_N)
        in_maps.append({"blob": blob})
    return in_maps


def kernel(**inputs) -> np.ndarray:
    nc = _build()
    in_maps = _prep_in_maps(inputs)
    res = bass_utils.run_bass_kernel_spmd(nc, in_maps, core_ids=list(range(R)))
    outs = [np.asarray(res.results[r]["out"]) for r in range(R)]
    feat = np.concatenate(
        [o[:NS * 1024].reshape(NS, 1024) for o in outs], axis=0)
    mbd = np.concatenate(
        [o[NS * 1024:].reshape(128, 32) for o in outs], axis=1)
    return np.concatenate([feat, mbd], axis=1).astype(np.float32)


if __name__ == "__main__":
    _build()
    print("build ok")
